# revision 54
# baseline (speedup 1.0000x reference)
"""CGCNN-style GNN message passing on 8 Trainium2 NeuronCores.

Sharding: data-parallel over graphs (4 graphs / core).  Each core holds its
4096 nodes and their 131072 in-edges entirely locally (edges never cross
graph boundaries).  Only BatchNorm batch statistics are all-reduced (one
[128,8] f32 AllReduce per layer: {sum agg, sum agg^2, sum agg*x, sum x,
sum x^2} lets both BN1 and BN2 affines be derived from a single exchange).

Device-side layout is feature-major ([128 features x nodes/edges]):
  - x kept feature-major (f32 + bf16 copies) and as per-graph node-major
    "stripes" (gather source: node m -> partition m%128, 256B rank m//128).
  - per-edge pre-activations a = W_col^T x[col] + w_d*dist + W_row^T x[row]
    accumulate on the PE into PSUM, weight-major per psum tile (each
    LDWEIGHTS feeds two back-to-back 512-col matmuls):
      * W_col term: rhs = broadcast AP view of x (col = edge//32)
      * dist term:  K=1 matmul, rhs = [1, 512] dist slices DMAd from DRAM
      * W_row term: rhs = gathered x columns (dma_gather transpose=True,
        all on SWDGE queue 0 -- concurrent desc-gen on multiple queues
        corrupts data on this ucode build, hardware-verified)
  - layer 0 needs no gathers: x0[row] = emb[z[row]] and z < 128, so the
    row term is brow^T onehot(z[row]) with the one-hot built on device
    (K=1 replication matmul + DVE is_equal against the partition index).
  - activations batched per gate to avoid ACT table thrash: all sigmoid
    tiles of a chunk, then all exp tiles, then one Ln(1+u) pass.
  - message product on DVE (bf16 2x), 32-edge segment sum via tensor_reduce.
  - x0 = emb[z] via one-hot matmul (host ships the [128, NN] one-hot of z).
"""

import os
import sys

sys.path.insert(0, "/opt/trn_rl_repo")

import numpy as np
import ml_dtypes

import concourse.bass as bass
import concourse.bacc as bacc
import concourse.mybir as mybir
import concourse.tile as tile

f32 = mybir.dt.float32
bf16 = mybir.dt.bfloat16
i16 = mybir.dt.int16
AF = mybir.ActivationFunctionType
OP = mybir.AluOpType

EPS = 1e-5


class Cfg:
    def __init__(self, G=32, S=1024, DEG=32, D=128, L=4, NCORE=8, CH=8192):
        self.G, self.S, self.DEG, self.D, self.L, self.NCORE = G, S, DEG, D, L, NCORE
        self.GP = G // NCORE            # graphs per core
        self.NN = self.GP * S           # nodes per core
        self.NE = self.NN * DEG         # edges per core
        self.CH = min(CH, self.NE)      # edge chunk
        self.NCH = self.NE // self.CH
        self.PT = 1024                  # psum tile cols (2 banks)
        self.NT = 512                   # matmul N per region (1 bank)
        assert self.NN % 128 == 0 and self.NE % self.CH == 0
        assert self.CH % self.PT == 0 and self.PT % self.NT == 0
        assert self.NE // self.PT == 128  # dist partition-major layout
        self.NTOT = self.NN * NCORE     # total nodes (BN denominator)


def wrap16(idx):
    """[n] -> [128, n/16] int16 wrapped layout for dma_gather index tensors."""
    a = np.asarray(idx, np.int16).reshape(-1, 16).T  # [16, n/16]
    return np.tile(a, (8, 1)).copy()                 # [128, n/16]


def build_nc(cfg, debug=False):
    NN, NE, D, L, CH = cfg.NN, cfg.NE, cfg.D, cfg.L, cfg.CH
    NT, PT, DEG, NCH = cfg.NT, cfg.PT, cfg.DEG, cfg.NCH
    TPC = CH // PT                       # psum tiles per chunk

    nc = bacc.Bacc("TRN2", target_bir_lowering=False, debug=False,
                   num_devices=cfg.NCORE, num_swdge_queues=4)

    def din(name, shape, dt):
        return nc.dram_tensor(name, shape, dt, kind="ExternalInput")

    emb_d = din("emb_t", [128, 128], f32)                  # emb padded [z, f]
    oh_d = din("oh_z", [128, NN], f32)                     # one-hot of z
    ridx_d = din("ridx", [128, NE // 16], i16)
    dist_d = din("dist_p", [128, NE // 128], bf16)         # row=global tile idx
    zrow_d = din("zrow_p", [128, NE // 128], bf16)         # z[row], same layout
    brow_d = din("brow", [2, 128, 128], bf16)              # emb_pad @ Wrow[0,g]
    iota_d = din("iota_p", [128, 1], f32)
    ones_d = din("ones_r", [1, 128], bf16)
    wcol_d = din("wcol", [L, 2, 128, 128], bf16)
    wrow_d = din("wrow", [L, 2, 128, 128], bf16)
    wdst_d = din("wdst", [L, 2, 1, 128], bf16)
    bias_d = din("bias", [L, 2, 128], f32)
    gc_d = din("gc_p", [L, 128], f32)
    gn_d = din("gn_p", [L, 128], f32)
    bnb_d = din("bnb_p", [L, 128], f32)
    w1_d = din("w1_p", [128, 128], f32)                    # pre-scaled by 1/S
    b1_d = din("b1_p", [128], f32)
    w2_d = din("w2_p", [128, 1], f32)
    b2_d = din("b2_p", [1, 1], f32)
    ident_d = din("ident", [128, 128], bf16)

    out_d = nc.dram_tensor("out4", [1, cfg.GP], f32, kind="ExternalOutput")
    if debug:
        xdbg_d = nc.dram_tensor("xdbg", [128, NN], f32, kind="ExternalOutput")
        adbg_d = nc.dram_tensor("adbg", [128, NN], f32, kind="ExternalOutput")

    groups = [list(range(cfg.NCORE))]
    NTOT_INV = 1.0 / float(cfg.NTOT)
    # Concurrent dma_gathers (multiple SWDGE queues) corrupt data on this
    # ucode build (hardware-verified: overlapping desc-gen shifts output
    # columns nondeterministically) -- keep every gather on queue 0, serial.
    QORDER = tuple(int(x) for x in os.environ.get("KQORDER", "0,0,0,0").split(","))

    with tile.TileContext(nc) as tc:
        with (
            tc.tile_pool(name="const", bufs=1) as cp,
            tc.tile_pool(name="xpool", bufs=2) as xp,
            tc.tile_pool(name="xbf", bufs=1) as xb,
            tc.tile_pool(name="node", bufs=1) as npo,
            tc.tile_pool(name="idxp", bufs=2) as ip,
            tc.tile_pool(name="distp", bufs=1) as dip,
            tc.tile_pool(name="gath", bufs=1) as gp,
            tc.tile_pool(name="acts", bufs=1) as ap_,
            tc.tile_pool(name="small", bufs=1) as sp_,
            tc.tile_pool(name="ps", bufs=3, space="PSUM") as pp,
            tc.tile_pool(name="pst", bufs=1, space="PSUM") as ppt,
            tc.tile_pool(name="dram", bufs=2, space="DRAM") as dp,
        ):
            # ---------------- constants ----------------
            emb_sb = cp.tile([128, 128], f32)
            nc.sync.dma_start(emb_sb[:], emb_d[:])
            ident_sb = cp.tile([128, 128], bf16)
            nc.sync.dma_start(ident_sb[:], ident_d[:])
            brow_sb = {}
            for g in range(2):
                t = cp.tile([128, 128], bf16, tag=f"br{g}")
                nc.sync.dma_start(t[:], brow_d[g])
                brow_sb[g] = t
            iota_sb = cp.tile([128, 1], f32)
            nc.sync.dma_start(iota_sb[:], iota_d[:])
            ones_sb = cp.tile([1, 128], bf16)
            nc.sync.dma_start(ones_sb[:], ones_d[:])
            wcol_sb, wrow_sb, wdst_sb, bias_sb = {}, {}, {}, {}
            for l in range(L):
                for g in range(2):
                    t = cp.tile([128, 128], bf16, tag=f"wc{l}{g}")
                    nc.sync.dma_start(t[:], wcol_d[l, g])
                    wcol_sb[l, g] = t
                    t = cp.tile([128, 128], bf16, tag=f"wr{l}{g}")
                    nc.sync.dma_start(t[:], wrow_d[l, g])
                    wrow_sb[l, g] = t
                    t = cp.tile([1, 128], bf16, tag=f"wd{l}{g}")
                    nc.sync.dma_start(t[:], wdst_d[l, g])
                    wdst_sb[l, g] = t
                    t = cp.tile([128, 1], f32, tag=f"bi{l}{g}")
                    nc.sync.dma_start(t[:], bias_d[l, g].rearrange("(p o) -> p o", o=1))
                    bias_sb[l, g] = t
            gc_sb, gn_sb, bnb_sb = {}, {}, {}
            for l in range(L):
                for nm, d_, dst in (("gc", gc_d, gc_sb), ("gn", gn_d, gn_sb),
                                    ("bb", bnb_d, bnb_sb)):
                    t = cp.tile([128, 1], f32, tag=f"{nm}{l}")
                    nc.sync.dma_start(t[:], d_[l].rearrange("(p o) -> p o", o=1))
                    dst[l] = t
            w1_sb = cp.tile([128, 128], f32)
            nc.sync.dma_start(w1_sb[:], w1_d[:])
            b1_sb = cp.tile([128, 1], f32)
            nc.sync.dma_start(b1_sb[:], b1_d[:].rearrange("(p o) -> p o", o=1))
            w2_sb = cp.tile([128, 1], f32)
            nc.sync.dma_start(w2_sb[:], w2_d[:])
            b2_sb = cp.tile([1, 1], f32)
            nc.sync.dma_start(b2_sb[:], b2_d[:])

            # ---------------- x0 = emb[z] via one-hot matmul ----------------

            def make_stripes_graph(src_bf, st, g4):
                """Graph g4's [128 f, 1024] block -> node-major stripes
                (node m -> partition m%128, byte range [(m//128)*256, +256)).
                4 transposes batched per PSUM tile, one copy each."""
                for j in range(2):
                    ptile = ppt.tile([128, 512], bf16, tag="tp")
                    for k in range(4):
                        r = j * 4 + k
                        nc.tensor.transpose(
                            ptile[:, k * 128:(k + 1) * 128],
                            src_bf[:, g4 * 1024 + r * 128:g4 * 1024 + (r + 1) * 128],
                            ident_sb[:])
                    nc.vector.tensor_copy(out=st[:, j * 512:(j + 1) * 512],
                                          in_=ptile[:])

            xT_f = xp.tile([128, NN], f32, tag="xf32")
            xT_bf = xb.tile([128, NN], bf16, tag="xbf")
            for t in range(NN // PT):
                ohc = npo.tile([128, PT], f32, tag=f"ohc{t % 2}")
                nc.sync.dma_start(ohc[:], oh_d[:, t * PT:(t + 1) * PT])
                ps0 = pp.tile([128, PT], f32, tag="ps")
                for u in range(PT // NT):
                    ou = slice(u * NT, (u + 1) * NT)
                    nc.tensor.matmul(ps0[:, ou], emb_sb[:], ohc[:, ou],
                                     start=True, stop=True)
                oc = slice(t * PT, (t + 1) * PT)
                nc.scalar.activation(out=xT_f[:, oc], in_=ps0[:],
                                     func=AF.Identity, bias=0.0, scale=1.0)
                nc.vector.tensor_copy(out=xT_bf[:, oc], in_=xT_f[:, oc])
            stripes = None   # layer 0 needs no gathers (one-hot z path)
            GPG = cfg.GP     # graphs per core (stripes are per-graph)

            def rsqrt1(v, tagp):
                """[128,1] var -> 1/sqrt(var+eps) with one Newton step."""
                nc.vector.tensor_scalar_add(out=v[:], in0=v[:], scalar1=EPS)
                s = sp_.tile([128, 1], f32, tag=f"s{tagp}")
                nc.scalar.sqrt(out=s[:], in_=v[:])
                r = sp_.tile([128, 1], f32, tag=f"r{tagp}")
                nc.vector.reciprocal(out=r[:], in_=s[:])
                a = sp_.tile([128, 1], f32, tag=f"a{tagp}")
                nc.vector.tensor_mul(out=a[:], in0=r[:], in1=r[:])
                nc.vector.tensor_mul(out=a[:], in0=v[:], in1=a[:])
                nc.vector.tensor_scalar(out=a[:], in0=a[:], scalar1=-0.5,
                                        scalar2=1.5, op0=OP.mult, op1=OP.add)
                nc.vector.tensor_mul(out=r[:], in0=r[:], in1=a[:])
                return r

            # ---------------- layers ----------------
            for l in range(L):
                agg = npo.tile([128, NN], f32, tag="agg")
                # node-level col terms A_c[g] = W_col^T x  ([128, NN] bf16):
                # the per-edge col contribution is constant within each
                # 32-edge group, so it is added into PSUM by one DVE op per
                # tile (broadcast view) instead of a K=128 matmul per region.
                ac_sb = {}
                for g in range(2):
                    act_ = xb.tile([128, NN], bf16, tag=f"ac{g}", name=f"ac{g}")
                    for t4 in range(NN // PT):
                        psA = pp.tile([128, PT], f32, tag="ps")
                        for u in range(PT // NT):
                            o = slice(t4 * PT + u * NT, t4 * PT + (u + 1) * NT)
                            nc.tensor.matmul(
                                psA[:, u * NT:(u + 1) * NT],
                                wcol_sb[l, g][:], xT_bf[:, o],
                                start=True, stop=True)
                        nc.vector.tensor_copy(
                            out=act_[:, t4 * PT:(t4 + 1) * PT], in_=psA[:])
                    ac_sb[g] = act_
                for c in range(NCH):
                    e0c = c * CH
                    if l > 0:
                        idxc = ip.tile([128, CH // 16], i16, tag=f"idx{c % 2}")
                        nc.sync.dma_start(
                            idxc[:], ridx_d[:, e0c // 16:(e0c + CH) // 16])
                        xg = gp.tile([128, CH], bf16, tag=f"xg{c % 2}")
                        # two half-gathers: the first half's tiles can start
                        # on the PE while the second half's descriptors are
                        # still being generated / drained
                        for h in range(4):
                            hs = slice(h * CH // 4, (h + 1) * CH // 4)
                            nc.gpsimd.dma_gather(
                                out_ap=xg[:, hs].rearrange(
                                    "p (a n) -> p a n", a=1),
                                in_ap=stripes[c // 4][:],
                                idxs_ap=idxc[:, h * CH // 64:(h + 1) * CH // 64],
                                num_idxs=CH // 4, num_idxs_reg=CH // 4,
                                elem_size=128,
                                transpose=True, sbuf_tokens_per_rank=128,
                                sbuf_free_dim_per_rank=256,
                                sbuf_free_dim_pad_per_rank=0, sbuf_byte_offset=0,
                                single_packet=False, queue_num=QORDER[c % 4])
                    sgf = ap_.tile([128, CH], bf16, tag="sgf")
                    usb = ap_.tile([128, CH], bf16, tag="usb")
                    if l == 0:
                        # layer-0 x0[row] term: one-hot of z[row] (values<128)
                        # built on device -- replicate zrow across partitions
                        # on the (otherwise idle) GPSIMD daisy chain, compare
                        # to the partition index on DVE (bf16 4x mode).
                        xg = gp.tile([128, CH], bf16, tag=f"xg{c % 2}")
                        zrep = gp.tile([128, CH], bf16, tag="zrep")
                        for t in range(TPC):
                            q = c * TPC + t
                            zr = dip.tile([1, PT], bf16, tag=f"zr{t % 2}")
                            nc.sync.dma_start(zr[:], zrow_d[q:q + 1, :])
                            nc.gpsimd.partition_broadcast(
                                zrep[:, t * PT:(t + 1) * PT], zr[0:1, :])
                            nc.vector.tensor_scalar(
                                out=xg[:, t * PT:(t + 1) * PT],
                                in0=zrep[:, t * PT:(t + 1) * PT],
                                scalar1=iota_sb[:], scalar2=None,
                                op0=OP.is_equal)
                    # gate-major: all f tiles (sigmoid set), then all s tiles
                    # (exp), then one Ln(1+u) pass -> 2 table loads per chunk.
                    dists = {}
                    for t in range(TPC):
                        q = c * TPC + t              # global psum-tile index
                        dc = dip.tile([1, PT], bf16, tag=f"dc{t}")
                        nc.sync.dma_start(dc[:], dist_d[q:q + 1, :])
                        dists[t] = dc
                    for g in range(2):
                        for t in range(TPC):
                            ps = pp.tile([128, PT], f32, tag="ps")
                            # weight-major over the two 512-regions so each
                            # LDWEIGHTS serves two back-to-back matmuls
                            for wi in range(2):
                                for u in range(PT // NT):
                                    ecl = t * PT + u * NT
                                    o = slice(ecl, ecl + NT)
                                    ou = slice(u * NT, (u + 1) * NT)
                                    if wi == 0:
                                        w = wdst_sb[l, g]
                                        rhs = dists[t][0:1, ou]
                                    else:
                                        w = (brow_sb[g] if l == 0
                                             else wrow_sb[l, g])
                                        rhs = xg[:, o]
                                    nc.tensor.matmul(
                                        ps[:, ou], w[:], rhs,
                                        start=(wi == 0), stop=(wi == 1))
                            # + col term (constant per 32-edge group)
                            n0 = (e0c + t * PT) // DEG
                            nn_ = PT // DEG
                            nc.vector.tensor_add(
                                out=ps[:].rearrange("p (n k) -> p n k", k=DEG),
                                in0=ps[:].rearrange("p (n k) -> p n k", k=DEG),
                                in1=(ac_sb[g][:, n0:n0 + nn_]
                                     .unsqueeze(2)
                                     .to_broadcast((128, nn_, DEG))))
                            oc = slice(t * PT, (t + 1) * PT)
                            if g == 0:
                                nc.scalar.activation(
                                    out=sgf[:, oc], in_=ps[:], func=AF.Sigmoid,
                                    bias=bias_sb[l, 0][:], scale=1.0)
                            else:
                                nc.scalar.activation(
                                    out=usb[:, oc], in_=ps[:], func=AF.Exp,
                                    bias=bias_sb[l, 1][:], scale=1.0)
                    # softplus tail: sp = ln(1 + u)   (in place)
                    nc.scalar.activation(out=usb[:], in_=usb[:], func=AF.Ln,
                                         bias=1.0, scale=1.0)
                    # message product (in place into sgf)
                    nc.vector.tensor_mul(out=sgf[:], in0=sgf[:], in1=usb[:])
                    # segment sum over DEG=32
                    nc.vector.tensor_reduce(
                        out=agg[:, e0c // DEG:(e0c + CH) // DEG],
                        in_=sgf[:].rearrange("p (n k) -> p n k", k=DEG),
                        axis=mybir.AxisListType.X, op=OP.add)

                # ---- BN stats: one AllReduce of [sum agg, sum agg^2,
                #      sum agg*x, sum x, sum x^2] ----
                st = sp_.tile([128, 8], f32, tag="st")
                nc.vector.tensor_reduce(out=st[:, 0:1], in_=agg[:],
                                        axis=mybir.AxisListType.X, op=OP.add)
                nc.vector.tensor_reduce(out=st[:, 3:4], in_=xT_f[:],
                                        axis=mybir.AxisListType.X, op=OP.add)
                NP4 = 4
                NQ = NN // NP4
                pq = sp_.tile([128, 3 * NP4], f32, tag="pq")
                scr = npo.tile([128, NQ], f32, tag="scratch")
                for q in range(NP4):
                    qs = slice(q * NQ, (q + 1) * NQ)
                    nc.vector.scalar_tensor_tensor(
                        out=scr[:], in0=agg[:, qs], scalar=0.0, in1=agg[:, qs],
                        op0=OP.add, op1=OP.mult, accum_out=pq[:, q:q + 1])
                    nc.vector.scalar_tensor_tensor(
                        out=scr[:], in0=agg[:, qs], scalar=0.0, in1=xT_f[:, qs],
                        op0=OP.add, op1=OP.mult,
                        accum_out=pq[:, NP4 + q:NP4 + q + 1])
                    nc.vector.scalar_tensor_tensor(
                        out=scr[:], in0=xT_f[:, qs], scalar=0.0, in1=xT_f[:, qs],
                        op0=OP.add, op1=OP.mult,
                        accum_out=pq[:, 2 * NP4 + q:2 * NP4 + q + 1])
                for k, col in ((0, 1), (1, 2), (2, 4)):
                    nc.vector.tensor_reduce(
                        out=st[:, col:col + 1],
                        in_=pq[:, k * NP4:(k + 1) * NP4],
                        axis=mybir.AxisListType.X, op=OP.add)

                cin = dp.tile([128, 8], f32, tag=f"ci{l}")
                cout = dp.tile([128, 8], f32, tag=f"co{l}")
                nc.sync.dma_start(cin[:], st[:])
                nc.gpsimd.collective_compute(
                    "AllReduce", OP.add, replica_groups=groups,
                    ins=[cin[:].opt()], outs=[cout[:].opt()])
                stg = sp_.tile([128, 8], f32, tag="sg")
                nc.sync.dma_start(stg[:], cout[:])

                # BN1: mu1/var1 from s1,s2 ; gsc = gc * rsqrt(var1+eps)
                mu1 = sp_.tile([128, 1], f32, tag="mu1")
                nc.vector.tensor_scalar_mul(out=mu1[:], in0=stg[:, 0:1],
                                            scalar1=NTOT_INV)
                v1 = sp_.tile([128, 1], f32, tag="v1")
                nc.vector.tensor_mul(out=v1[:], in0=mu1[:], in1=mu1[:])
                nc.vector.scalar_tensor_tensor(
                    out=v1[:], in0=stg[:, 1:2], scalar=NTOT_INV, in1=v1[:],
                    op0=OP.mult, op1=OP.subtract)
                r1 = rsqrt1(v1, "1")
                gsc = sp_.tile([128, 1], f32, tag="gsc")
                nc.vector.tensor_mul(out=gsc[:], in0=gc_sb[l][:], in1=r1[:])

                # BN2 stats derived: sum_xmid = gsc*s1 + s4,
                # sumsq_xmid = gsc^2*s2 + 2*gsc*s3 + s5
                sm2 = sp_.tile([128, 1], f32, tag="sm2")
                nc.vector.scalar_tensor_tensor(
                    out=sm2[:], in0=stg[:, 0:1], scalar=gsc[:], in1=stg[:, 3:4],
                    op0=OP.mult, op1=OP.add)
                # sq2 = gsc*(gsc*s2 + 2*s3) + s5
                sq2 = sp_.tile([128, 1], f32, tag="sq2")
                t1 = sp_.tile([128, 1], f32, tag="t1")
                nc.vector.tensor_scalar_mul(out=t1[:], in0=stg[:, 2:3], scalar1=2.0)
                nc.vector.scalar_tensor_tensor(
                    out=sq2[:], in0=stg[:, 1:2], scalar=gsc[:], in1=t1[:],
                    op0=OP.mult, op1=OP.add)
                nc.vector.scalar_tensor_tensor(
                    out=sq2[:], in0=sq2[:], scalar=gsc[:], in1=stg[:, 4:5],
                    op0=OP.mult, op1=OP.add)
                mu2 = sp_.tile([128, 1], f32, tag="mu2")
                nc.vector.tensor_scalar_mul(out=mu2[:], in0=sm2[:],
                                            scalar1=NTOT_INV)
                v2 = sp_.tile([128, 1], f32, tag="v2")
                nc.vector.tensor_mul(out=v2[:], in0=mu2[:], in1=mu2[:])
                nc.vector.scalar_tensor_tensor(
                    out=v2[:], in0=sq2[:], scalar=NTOT_INV, in1=v2[:],
                    op0=OP.mult, op1=OP.subtract)
                r2 = rsqrt1(v2, "2")
                sc2 = sp_.tile([128, 1], f32, tag="sc2")
                nc.vector.tensor_mul(out=sc2[:], in0=gn_sb[l][:], in1=r2[:])
                b2t = sp_.tile([128, 1], f32, tag="b2t")
                nc.vector.tensor_mul(out=b2t[:], in0=sc2[:], in1=mu2[:])
                nc.vector.tensor_sub(out=b2t[:], in0=bnb_sb[l][:], in1=b2t[:])

                # x_mid = gsc*agg + x (in place into agg; BN1 shift cancels
                # in BN2), then x_new = relu(sc2*x_mid + b2t)
                nc.vector.scalar_tensor_tensor(
                    out=agg[:], in0=agg[:], scalar=gsc[:], in1=xT_f[:],
                    op0=OP.mult, op1=OP.add)
                xT_f = xp.tile([128, NN], f32, tag="xf32")
                xT_bf = xb.tile([128, NN], bf16, tag="xbf")
                if l < L - 1:
                    stripes = [xb.tile([128, 1024], bf16, tag=f"str{g4}", name=f"strt{g4}")
                               for g4 in range(GPG)]
                # per-graph tail so layer l+1's first gathers start while
                # later graphs are still being transposed
                for g4 in range(GPG):
                    sl = slice(g4 * 1024, (g4 + 1) * 1024)
                    nc.scalar.activation(out=xT_f[:, sl], in_=agg[:, sl],
                                         func=AF.Relu, bias=b2t[:],
                                         scale=sc2[:])
                    nc.vector.tensor_copy(out=xT_bf[:, sl], in_=xT_f[:, sl])
                    if l < L - 1:
                        make_stripes_graph(xT_bf, stripes[g4], g4)

            if debug:
                nc.sync.dma_start(xdbg_d[:], xT_f[:])
                nc.sync.dma_start(adbg_d[:], agg[:])

            # ---------------- readout ----------------
            gsum = sp_.tile([128, cfg.GP], f32, tag="gsum")
            nc.vector.tensor_reduce(
                out=gsum[:], in_=xT_f[:].rearrange("p (g s) -> p g s", s=cfg.S),
                axis=mybir.AxisListType.X, op=OP.add)
            ph = ppt.tile([128, cfg.GP], f32, tag="tp")
            nc.tensor.matmul(ph[:], w1_sb[:], gsum[:], start=True, stop=True)
            h = sp_.tile([128, cfg.GP], f32, tag="h")
            nc.scalar.activation(out=h[:], in_=ph[:], func=AF.Relu,
                                 bias=b1_sb[:], scale=1.0)
            po = ppt.tile([1, cfg.GP], f32, tag="tp2")
            nc.tensor.matmul(po[:], w2_sb[:], h[:], start=True, stop=True)
            osb = sp_.tile([1, cfg.GP], f32, tag="osb")
            nc.scalar.activation(out=osb[:], in_=po[:], func=AF.Identity,
                                 bias=b2_sb[:], scale=1.0)
            nc.sync.dma_start(out_d[:], osb[:])

    nc.compile()
    return nc


def prep_inputs(inputs, cfg):
    """Full inputs -> per-core input maps (host-side sharding + layout)."""
    bfc = lambda a: np.asarray(a, np.float32).astype(ml_dtypes.bfloat16)
    z = np.asarray(inputs["z"])
    pos = np.asarray(inputs["pos"], np.float32)
    ei = np.asarray(inputs["edge_index"])
    row, col = ei[0].astype(np.int64), ei[1].astype(np.int64)
    Wf = np.asarray(inputs["Wf"], np.float32)
    Ws = np.asarray(inputs["Ws"], np.float32)
    bf_ = np.asarray(inputs["bf"], np.float32)
    bs_ = np.asarray(inputs["bs"], np.float32)
    gc = np.asarray(inputs["gc"], np.float32)
    gn = np.asarray(inputs["gn"], np.float32)
    bnb = np.asarray(inputs["bn_b"], np.float32)
    W1 = np.asarray(inputs["W1"], np.float32)
    b1 = np.asarray(inputs["b1"], np.float32)
    W2 = np.asarray(inputs["W2"], np.float32)
    b2 = np.asarray(inputs["b2"], np.float32)
    emb = np.asarray(inputs["emb"], np.float32)

    D, L = cfg.D, cfg.L
    # lhsT for the one-hot matmul: out[f, n] = sum_p emb_t[p, f] * oh[p, n]
    emb_t = np.zeros((128, 128), np.float32)
    emb_t[:emb.shape[0], :] = emb

    wcol = np.stack([np.stack([bfc(Wf[l, :D]), bfc(Ws[l, :D])]) for l in range(L)])
    wrow = np.stack([np.stack([bfc(Wf[l, D:2 * D]), bfc(Ws[l, D:2 * D])])
                     for l in range(L)])
    wdst = np.stack([np.stack([bfc(Wf[l, 2 * D:2 * D + 1]),
                               bfc(Ws[l, 2 * D:2 * D + 1])]) for l in range(L)])
    biases = np.stack([np.stack([bf_[l], bs_[l]]) for l in range(L)])

    dist_full = np.sqrt(
        ((pos[row] - pos[col]) ** 2).sum(-1)).astype(np.float32)  # [E]

    brow = np.stack([bfc(emb_t @ Wf[0, D:2 * D]), bfc(emb_t @ Ws[0, D:2 * D])])
    shared = dict(
        emb_t=emb_t, wcol=wcol, wrow=wrow, wdst=wdst, bias=biases,
        gc_p=gc, gn_p=gn, bnb_p=bnb,
        w1_p=(W1 / cfg.S).astype(np.float32),
        b1_p=b1, w2_p=W2, b2_p=b2.reshape(1, 1),
        ident=np.eye(128, dtype=np.float32).astype(ml_dtypes.bfloat16),
        brow=brow,
        iota_p=np.arange(128, dtype=np.float32).reshape(128, 1),
        ones_r=np.ones((1, 128), np.float32).astype(ml_dtypes.bfloat16),
    )

    maps = []
    for c in range(cfg.NCORE):
        n0, n1 = c * cfg.NN, (c + 1) * cfg.NN
        e0, e1 = c * cfg.NE, (c + 1) * cfg.NE
        zc = z[n0:n1]
        rl = row[e0:e1] - n0
        assert rl.min() >= 0 and rl.max() < cfg.NN, "edges cross core boundary"
        # graph-local indices (gather sources are per-graph stripe tiles)
        rl = rl - (np.arange(cfg.NE) // (cfg.S * cfg.DEG)) * cfg.S
        assert rl.min() >= 0 and rl.max() < cfg.S, "edges cross graph boundary"
        oh = (zc[None, :] == np.arange(128)[:, None])
        m = dict(shared)
        m.update(
            oh_z=oh.astype(np.float32),
            ridx=wrap16(rl),
            dist_p=bfc(dist_full[e0:e1].reshape(128, cfg.NE // 128)),
            zrow_p=bfc(z[row[e0:e1]].reshape(128, cfg.NE // 128)),
        )
        maps.append(m)
    return maps


_CACHE = {}


def make_runner(nc, n_cores):
    """Build a reusable jitted PJRT executable for `nc` (one NEFF compile +
    load; repeat calls only transfer inputs and execute)."""
    import jax
    from jax.sharding import Mesh, PartitionSpec
    from jax.experimental.shard_map import shard_map
    from concourse.bass2jax import (_bass_exec_p, install_neuronx_cc_hook,
                                    partition_id_tensor)
    import concourse.mybir as mybir

    install_neuronx_cc_hook()
    partition_name = (nc.partition_id_tensor.name
                      if nc.partition_id_tensor else None)
    in_names, out_names, out_avals, zero_outs = [], [], [], []
    for alloc in nc.m.functions[0].allocations:
        if not isinstance(alloc, mybir.MemoryLocationSet):
            continue
        name = alloc.memorylocations[0].name
        if alloc.kind == "ExternalInput":
            if name != partition_name:
                in_names.append(name)
        elif alloc.kind == "ExternalOutput":
            shape = tuple(alloc.tensor_shape)
            dtype = mybir.dt.np(alloc.dtype)
            out_names.append(name)
            out_avals.append(jax.core.ShapedArray(shape, dtype))
            zero_outs.append(np.zeros(shape, dtype))
    n_params = len(in_names)
    n_outs = len(out_avals)
    all_in_names = list(in_names) + list(out_names)
    if partition_name is not None:
        all_in_names.append(partition_name)
    donate = tuple(range(n_params, n_params + n_outs))

    def _body(*args):
        operands = list(args)
        if partition_name is not None:
            operands.append(partition_id_tensor())
        outs = _bass_exec_p.bind(
            *operands, out_avals=tuple(out_avals),
            in_names=tuple(all_in_names), out_names=tuple(out_names),
            lowering_input_output_aliases=(), sim_require_finite=True,
            sim_require_nnan=True, nc=nc)
        return tuple(outs)

    devices = jax.devices()[:n_cores]
    mesh = Mesh(np.asarray(devices), ("core",))
    in_specs = (PartitionSpec("core"),) * (n_params + n_outs)
    out_specs = (PartitionSpec("core"),) * n_outs
    sharded = jax.jit(
        shard_map(_body, mesh=mesh, in_specs=in_specs, out_specs=out_specs,
                  check_rep=False),
        donate_argnums=donate, keep_unused=True)

    def run(maps, device_inputs=None):
        if device_inputs is None:
            device_inputs = stage(maps)
        concat_zeros = [
            np.zeros((n_cores * z.shape[0], *z.shape[1:]), z.dtype)
            for z in zero_outs]
        out_arrs = sharded(*device_inputs, *concat_zeros)
        return [
            {name: np.asarray(out_arrs[i]).reshape(n_cores, *out_avals[i].shape)[c]
             for i, name in enumerate(out_names)}
            for c in range(n_cores)]

    def stage(maps):
        from jax.sharding import NamedSharding
        sh = NamedSharding(mesh, PartitionSpec("core"))
        return [
            jax.device_put(
                np.concatenate([np.asarray(maps[c][nm])
                                for c in range(n_cores)], axis=0), sh)
            for nm in in_names]

    run.stage = stage
    return run


def _get_nc(cfg_key=()):
    if cfg_key not in _CACHE:
        cfg = Cfg()
        nc = build_nc(cfg)
        runner = make_runner(nc, cfg.NCORE)
        _CACHE[cfg_key] = (cfg, nc, runner)
    return _CACHE[cfg_key]


def kernel(**inputs):
    cfg, nc, runner = _get_nc()
    # structural preconditions from the generator: edges grouped by target,
    # exactly DEG edges per node; batch = repeat(arange(G), S)
    ei = np.asarray(inputs["edge_index"])
    N = cfg.NN * cfg.NCORE
    assert np.array_equal(ei[1], np.repeat(np.arange(N), cfg.DEG)), \
        "edge_index[1] must be repeat(arange(N), DEG)"
    maps = prep_inputs(inputs, cfg)
    results = runner(maps)
    outs = [results[c]["out4"].reshape(-1) for c in range(cfg.NCORE)]
    return np.concatenate(outs).astype(np.float32)


if __name__ == "__main__":
    cfg = Cfg()
    nc = build_nc(cfg)
    print("built + compiled OK")


# revision 55
# speedup vs baseline: 1.0392x; 1.0392x over previous
"""CGCNN-style GNN message passing on 8 Trainium2 NeuronCores.

Sharding: data-parallel over graphs (4 graphs / core).  Each core holds its
4096 nodes and their 131072 in-edges entirely locally (edges never cross
graph boundaries).  Only BatchNorm batch statistics are all-reduced (one
[128,8] f32 AllReduce per layer: {sum agg, sum agg^2, sum agg*x, sum x,
sum x^2} lets both BN1 and BN2 affines be derived from a single exchange).

Device-side layout is feature-major ([128 features x nodes/edges]):
  - x kept feature-major (f32 + bf16 copies) and as per-graph node-major
    "stripes" (gather source: node m -> partition m%128, 256B rank m//128).
  - per-edge pre-activations a = W_col^T x[col] + w_d*dist + W_row^T x[row]
    accumulate on the PE into PSUM, weight-major per psum tile (each
    LDWEIGHTS feeds two back-to-back 512-col matmuls):
      * W_col term: rhs = broadcast AP view of x (col = edge//32)
      * dist term:  K=1 matmul, rhs = [1, 512] dist slices DMAd from DRAM
      * W_row term: rhs = gathered x columns (dma_gather transpose=True,
        all on SWDGE queue 0 -- concurrent desc-gen on multiple queues
        corrupts data on this ucode build, hardware-verified)
  - layer 0 needs no gathers: x0[row] = emb[z[row]] and z < 128, so the
    row term is brow^T onehot(z[row]) with the one-hot built on device
    (K=1 replication matmul + DVE is_equal against the partition index).
  - activations batched per gate to avoid ACT table thrash: all sigmoid
    tiles of a chunk, then all exp tiles, then one Ln(1+u) pass.
  - message product on DVE (bf16 2x), 32-edge segment sum via tensor_reduce.
  - x0 = emb[z] via one-hot matmul (host ships the [128, NN] one-hot of z).
"""

import os
import sys

sys.path.insert(0, "/opt/trn_rl_repo")

import numpy as np
import ml_dtypes

import concourse.bass as bass
import concourse.bacc as bacc
import concourse.mybir as mybir
import concourse.tile as tile

f32 = mybir.dt.float32
bf16 = mybir.dt.bfloat16
i16 = mybir.dt.int16
AF = mybir.ActivationFunctionType
OP = mybir.AluOpType

EPS = 1e-5


class Cfg:
    def __init__(self, G=32, S=1024, DEG=32, D=128, L=4, NCORE=8, CH=8192):
        self.G, self.S, self.DEG, self.D, self.L, self.NCORE = G, S, DEG, D, L, NCORE
        self.GP = G // NCORE            # graphs per core
        self.NN = self.GP * S           # nodes per core
        self.NE = self.NN * DEG         # edges per core
        self.CH = min(CH, self.NE)      # edge chunk
        self.NCH = self.NE // self.CH
        self.PT = 1024                  # psum tile cols (2 banks)
        self.NT = 512                   # matmul N per region (1 bank)
        assert self.NN % 128 == 0 and self.NE % self.CH == 0
        assert self.CH % self.PT == 0 and self.PT % self.NT == 0
        assert self.NE // self.PT == 128  # dist partition-major layout
        self.NTOT = self.NN * NCORE     # total nodes (BN denominator)


def wrap16(idx):
    """[n] -> [128, n/16] int16 wrapped layout for dma_gather index tensors."""
    a = np.asarray(idx, np.int16).reshape(-1, 16).T  # [16, n/16]
    return np.tile(a, (8, 1)).copy()                 # [128, n/16]


def build_nc(cfg, debug=False):
    NN, NE, D, L, CH = cfg.NN, cfg.NE, cfg.D, cfg.L, cfg.CH
    NT, PT, DEG, NCH = cfg.NT, cfg.PT, cfg.DEG, cfg.NCH
    TPC = CH // PT                       # psum tiles per chunk

    nc = bacc.Bacc("TRN2", target_bir_lowering=False, debug=False,
                   num_devices=cfg.NCORE, num_swdge_queues=4)

    def din(name, shape, dt):
        return nc.dram_tensor(name, shape, dt, kind="ExternalInput")

    emb_d = din("emb_t", [128, 128], f32)                  # emb padded [z, f]
    oh_d = din("oh_z", [128, NN], f32)                     # one-hot of z
    ridx_d = din("ridx", [128, NE // 16], i16)
    dist_d = din("dist_p", [128, NE // 128], bf16)         # row=global tile idx
    zrow_d = din("zrow_p", [128, NE // 128], bf16)         # z[row], same layout
    brow_d = din("brow", [2, 128, 128], bf16)              # emb_pad @ Wrow[0,g]
    iota_d = din("iota_p", [128, 1], f32)
    ones_d = din("ones_r", [1, 128], bf16)
    wcol_d = din("wcol", [L, 2, 128, 128], bf16)
    wrow_d = din("wrow", [L, 2, 128, 128], bf16)
    wdst_d = din("wdst", [L, 2, 1, 128], bf16)
    bias_d = din("bias", [L, 2, 128], f32)
    gc_d = din("gc_p", [L, 128], f32)
    gn_d = din("gn_p", [L, 128], f32)
    bnb_d = din("bnb_p", [L, 128], f32)
    w1_d = din("w1_p", [128, 128], f32)                    # pre-scaled by 1/S
    b1_d = din("b1_p", [128], f32)
    w2_d = din("w2_p", [128, 1], f32)
    b2_d = din("b2_p", [1, 1], f32)
    ident_d = din("ident", [128, 128], bf16)

    out_d = nc.dram_tensor("out4", [1, cfg.GP], f32, kind="ExternalOutput")
    if debug:
        xdbg_d = nc.dram_tensor("xdbg", [128, NN], f32, kind="ExternalOutput")
        adbg_d = nc.dram_tensor("adbg", [128, NN], f32, kind="ExternalOutput")

    groups = [list(range(cfg.NCORE))]
    NTOT_INV = 1.0 / float(cfg.NTOT)
    # Concurrent dma_gathers (multiple SWDGE queues) corrupt data on this
    # ucode build (hardware-verified: overlapping desc-gen shifts output
    # columns nondeterministically) -- keep every gather on queue 0, serial.
    QORDER = tuple(int(x) for x in os.environ.get("KQORDER", "0,0,0,0").split(","))

    with tile.TileContext(nc) as tc:
        with (
            tc.tile_pool(name="const", bufs=1) as cp,
            tc.tile_pool(name="xpool", bufs=2) as xp,
            tc.tile_pool(name="xbf", bufs=1) as xb,
            tc.tile_pool(name="node", bufs=1) as npo,
            tc.tile_pool(name="idxp", bufs=2) as ip,
            tc.tile_pool(name="distp", bufs=1) as dip,
            tc.tile_pool(name="gath", bufs=1) as gp,
            tc.tile_pool(name="acts", bufs=1) as ap_,
            tc.tile_pool(name="small", bufs=1) as sp_,
            tc.tile_pool(name="ps", bufs=3, space="PSUM") as pp,
            tc.tile_pool(name="pst", bufs=1, space="PSUM") as ppt,
            tc.tile_pool(name="dram", bufs=2, space="DRAM") as dp,
        ):
            # ---------------- constants ----------------
            emb_sb = cp.tile([128, 128], f32)
            nc.sync.dma_start(emb_sb[:], emb_d[:])
            ident_sb = cp.tile([128, 128], bf16)
            nc.sync.dma_start(ident_sb[:], ident_d[:])
            brow_sb = {}
            for g in range(2):
                t = cp.tile([128, 128], bf16, tag=f"br{g}")
                nc.sync.dma_start(t[:], brow_d[g])
                brow_sb[g] = t
            iota_sb = cp.tile([128, 1], f32)
            nc.sync.dma_start(iota_sb[:], iota_d[:])
            ones_sb = cp.tile([1, 128], bf16)
            nc.sync.dma_start(ones_sb[:], ones_d[:])
            wcol_sb, wrow_sb, wdst_sb, bias_sb = {}, {}, {}, {}
            for l in range(L):
                for g in range(2):
                    t = cp.tile([128, 128], bf16, tag=f"wc{l}{g}")
                    nc.sync.dma_start(t[:], wcol_d[l, g])
                    wcol_sb[l, g] = t
                    t = cp.tile([128, 128], bf16, tag=f"wr{l}{g}")
                    nc.sync.dma_start(t[:], wrow_d[l, g])
                    wrow_sb[l, g] = t
                    t = cp.tile([1, 128], bf16, tag=f"wd{l}{g}")
                    nc.sync.dma_start(t[:], wdst_d[l, g])
                    wdst_sb[l, g] = t
                    t = cp.tile([128, 1], f32, tag=f"bi{l}{g}")
                    nc.sync.dma_start(t[:], bias_d[l, g].rearrange("(p o) -> p o", o=1))
                    bias_sb[l, g] = t
            gc_sb, gn_sb, bnb_sb = {}, {}, {}
            for l in range(L):
                for nm, d_, dst in (("gc", gc_d, gc_sb), ("gn", gn_d, gn_sb),
                                    ("bb", bnb_d, bnb_sb)):
                    t = cp.tile([128, 1], f32, tag=f"{nm}{l}")
                    nc.sync.dma_start(t[:], d_[l].rearrange("(p o) -> p o", o=1))
                    dst[l] = t
            w1_sb = cp.tile([128, 128], f32)
            nc.sync.dma_start(w1_sb[:], w1_d[:])
            b1_sb = cp.tile([128, 1], f32)
            nc.sync.dma_start(b1_sb[:], b1_d[:].rearrange("(p o) -> p o", o=1))
            w2_sb = cp.tile([128, 1], f32)
            nc.sync.dma_start(w2_sb[:], w2_d[:])
            b2_sb = cp.tile([1, 1], f32)
            nc.sync.dma_start(b2_sb[:], b2_d[:])

            # ---------------- x0 = emb[z] via one-hot matmul ----------------

            def make_stripes_graph(src_bf, st, g4):
                """Graph g4's [128 f, 1024] block -> node-major stripes
                (node m -> partition m%128, byte range [(m//128)*256, +256)).
                4 transposes batched per PSUM tile, one copy each."""
                for j in range(2):
                    ptile = ppt.tile([128, 512], bf16, tag="tp")
                    for k in range(4):
                        r = j * 4 + k
                        nc.tensor.transpose(
                            ptile[:, k * 128:(k + 1) * 128],
                            src_bf[:, g4 * 1024 + r * 128:g4 * 1024 + (r + 1) * 128],
                            ident_sb[:])
                    nc.vector.tensor_copy(out=st[:, j * 512:(j + 1) * 512],
                                          in_=ptile[:])

            xT_f = xp.tile([128, NN], f32, tag="xf32")
            xT_bf = xb.tile([128, NN], bf16, tag="xbf")
            for t in range(NN // PT):
                ohc = npo.tile([128, PT], f32, tag=f"ohc{t % 2}")
                nc.sync.dma_start(ohc[:], oh_d[:, t * PT:(t + 1) * PT])
                ps0 = pp.tile([128, PT], f32, tag="ps")
                for u in range(PT // NT):
                    ou = slice(u * NT, (u + 1) * NT)
                    nc.tensor.matmul(ps0[:, ou], emb_sb[:], ohc[:, ou],
                                     start=True, stop=True)
                oc = slice(t * PT, (t + 1) * PT)
                nc.scalar.activation(out=xT_f[:, oc], in_=ps0[:],
                                     func=AF.Identity, bias=0.0, scale=1.0)
                nc.vector.tensor_copy(out=xT_bf[:, oc], in_=xT_f[:, oc])
            stripes = None   # layer 0 needs no gathers (one-hot z path)
            GPG = cfg.GP     # graphs per core (stripes are per-graph)

            def rsqrt1(v, tagp):
                """[128,1] var -> 1/sqrt(var+eps) with one Newton step."""
                nc.vector.tensor_scalar_add(out=v[:], in0=v[:], scalar1=EPS)
                s = sp_.tile([128, 1], f32, tag=f"s{tagp}")
                nc.scalar.sqrt(out=s[:], in_=v[:])
                r = sp_.tile([128, 1], f32, tag=f"r{tagp}")
                nc.vector.reciprocal(out=r[:], in_=s[:])
                a = sp_.tile([128, 1], f32, tag=f"a{tagp}")
                nc.vector.tensor_mul(out=a[:], in0=r[:], in1=r[:])
                nc.vector.tensor_mul(out=a[:], in0=v[:], in1=a[:])
                nc.vector.tensor_scalar(out=a[:], in0=a[:], scalar1=-0.5,
                                        scalar2=1.5, op0=OP.mult, op1=OP.add)
                nc.vector.tensor_mul(out=r[:], in0=r[:], in1=a[:])
                return r

            # ---------------- layers ----------------
            for l in range(L):
                agg = npo.tile([128, NN], f32, tag="agg")
                # node-level col terms A_c[g] = W_col^T x  ([128, NN] bf16):
                # the per-edge col contribution is constant within each
                # 32-edge group, so it is added into PSUM by one DVE op per
                # tile (broadcast view) instead of a K=128 matmul per region.
                ac_sb = {}
                for g in range(2):
                    act_ = xb.tile([128, NN], bf16, tag=f"ac{g}", name=f"ac{g}")
                    for t4 in range(NN // PT):
                        psA = pp.tile([128, PT], f32, tag="ps")
                        for u in range(PT // NT):
                            o = slice(t4 * PT + u * NT, t4 * PT + (u + 1) * NT)
                            nc.tensor.matmul(
                                psA[:, u * NT:(u + 1) * NT],
                                wcol_sb[l, g][:], xT_bf[:, o],
                                start=True, stop=True)
                        nc.vector.tensor_copy(
                            out=act_[:, t4 * PT:(t4 + 1) * PT], in_=psA[:])
                    ac_sb[g] = act_
                for c in range(NCH):
                    e0c = c * CH
                    if l > 0:
                        idxc = ip.tile([128, CH // 16], i16, tag=f"idx{c % 2}")
                        nc.sync.dma_start(
                            idxc[:], ridx_d[:, e0c // 16:(e0c + CH) // 16])
                        xg = gp.tile([128, CH], bf16, tag=f"xg{c % 2}")
                        # two half-gathers: the first half's tiles can start
                        # on the PE while the second half's descriptors are
                        # still being generated / drained
                        for h in range(2):
                            hs = slice(h * CH // 2, (h + 1) * CH // 2)
                            nc.gpsimd.dma_gather(
                                out_ap=xg[:, hs].rearrange(
                                    "p (a n) -> p a n", a=1),
                                in_ap=stripes[c // 4][:],
                                idxs_ap=idxc[:, h * CH // 32:(h + 1) * CH // 32],
                                num_idxs=CH // 2, num_idxs_reg=CH // 2,
                                elem_size=128,
                                transpose=True, sbuf_tokens_per_rank=128,
                                sbuf_free_dim_per_rank=256,
                                sbuf_free_dim_pad_per_rank=0, sbuf_byte_offset=0,
                                single_packet=False, queue_num=QORDER[c % 4])
                    sgf = ap_.tile([128, CH], bf16, tag="sgf")
                    usb = ap_.tile([128, CH], bf16, tag="usb")
                    if l == 0:
                        # layer-0 x0[row] term: one-hot of z[row] (values<128)
                        # built on device -- replicate zrow across partitions
                        # on the (otherwise idle) GPSIMD daisy chain, compare
                        # to the partition index on DVE (bf16 4x mode).
                        xg = gp.tile([128, CH], bf16, tag=f"xg{c % 2}")
                        zrep = gp.tile([128, CH], bf16, tag="zrep")
                        for t in range(TPC):
                            q = c * TPC + t
                            zr = dip.tile([1, PT], bf16, tag=f"zr{t % 2}")
                            nc.sync.dma_start(zr[:], zrow_d[q:q + 1, :])
                            nc.gpsimd.partition_broadcast(
                                zrep[:, t * PT:(t + 1) * PT], zr[0:1, :])
                            nc.vector.tensor_scalar(
                                out=xg[:, t * PT:(t + 1) * PT],
                                in0=zrep[:, t * PT:(t + 1) * PT],
                                scalar1=iota_sb[:], scalar2=None,
                                op0=OP.is_equal)
                    # gate-major: all f tiles (sigmoid set), then all s tiles
                    # (exp), then one Ln(1+u) pass -> 2 table loads per chunk.
                    dists = {}
                    for t in range(TPC):
                        q = c * TPC + t              # global psum-tile index
                        dc = dip.tile([1, PT], bf16, tag=f"dc{t}")
                        nc.sync.dma_start(dc[:], dist_d[q:q + 1, :])
                        dists[t] = dc
                    for g in range(2):
                        for t in range(TPC):
                            ps = pp.tile([128, PT], f32, tag="ps")
                            # weight-major over the two 512-regions so each
                            # LDWEIGHTS serves two back-to-back matmuls
                            for wi in range(2):
                                for u in range(PT // NT):
                                    ecl = t * PT + u * NT
                                    o = slice(ecl, ecl + NT)
                                    ou = slice(u * NT, (u + 1) * NT)
                                    if wi == 0:
                                        w = wdst_sb[l, g]
                                        rhs = dists[t][0:1, ou]
                                    else:
                                        w = (brow_sb[g] if l == 0
                                             else wrow_sb[l, g])
                                        rhs = xg[:, o]
                                    nc.tensor.matmul(
                                        ps[:, ou], w[:], rhs,
                                        start=(wi == 0), stop=(wi == 1))
                            # + col term (constant per 32-edge group)
                            n0 = (e0c + t * PT) // DEG
                            nn_ = PT // DEG
                            nc.vector.tensor_add(
                                out=ps[:].rearrange("p (n k) -> p n k", k=DEG),
                                in0=ps[:].rearrange("p (n k) -> p n k", k=DEG),
                                in1=(ac_sb[g][:, n0:n0 + nn_]
                                     .unsqueeze(2)
                                     .to_broadcast((128, nn_, DEG))))
                            oc = slice(t * PT, (t + 1) * PT)
                            if g == 0:
                                nc.scalar.activation(
                                    out=sgf[:, oc], in_=ps[:], func=AF.Sigmoid,
                                    bias=bias_sb[l, 0][:], scale=1.0)
                            else:
                                nc.scalar.activation(
                                    out=usb[:, oc], in_=ps[:], func=AF.Exp,
                                    bias=bias_sb[l, 1][:], scale=1.0)
                    # softplus tail: sp = ln(1 + u)   (in place)
                    nc.scalar.activation(out=usb[:], in_=usb[:], func=AF.Ln,
                                         bias=1.0, scale=1.0)
                    # message product (in place into sgf)
                    nc.vector.tensor_mul(out=sgf[:], in0=sgf[:], in1=usb[:])
                    # segment sum over DEG=32
                    nc.vector.tensor_reduce(
                        out=agg[:, e0c // DEG:(e0c + CH) // DEG],
                        in_=sgf[:].rearrange("p (n k) -> p n k", k=DEG),
                        axis=mybir.AxisListType.X, op=OP.add)

                # ---- BN stats: one AllReduce of [sum agg, sum agg^2,
                #      sum agg*x, sum x, sum x^2] ----
                st = sp_.tile([128, 8], f32, tag="st")
                nc.vector.tensor_reduce(out=st[:, 0:1], in_=agg[:],
                                        axis=mybir.AxisListType.X, op=OP.add)
                nc.vector.tensor_reduce(out=st[:, 3:4], in_=xT_f[:],
                                        axis=mybir.AxisListType.X, op=OP.add)
                NP4 = 4
                NQ = NN // NP4
                pq = sp_.tile([128, 3 * NP4], f32, tag="pq")
                scr = npo.tile([128, NQ], f32, tag="scratch")
                for q in range(NP4):
                    qs = slice(q * NQ, (q + 1) * NQ)
                    nc.vector.scalar_tensor_tensor(
                        out=scr[:], in0=agg[:, qs], scalar=0.0, in1=agg[:, qs],
                        op0=OP.add, op1=OP.mult, accum_out=pq[:, q:q + 1])
                    nc.vector.scalar_tensor_tensor(
                        out=scr[:], in0=agg[:, qs], scalar=0.0, in1=xT_f[:, qs],
                        op0=OP.add, op1=OP.mult,
                        accum_out=pq[:, NP4 + q:NP4 + q + 1])
                    nc.vector.scalar_tensor_tensor(
                        out=scr[:], in0=xT_f[:, qs], scalar=0.0, in1=xT_f[:, qs],
                        op0=OP.add, op1=OP.mult,
                        accum_out=pq[:, 2 * NP4 + q:2 * NP4 + q + 1])
                for k, col in ((0, 1), (1, 2), (2, 4)):
                    nc.vector.tensor_reduce(
                        out=st[:, col:col + 1],
                        in_=pq[:, k * NP4:(k + 1) * NP4],
                        axis=mybir.AxisListType.X, op=OP.add)

                cin = dp.tile([128, 8], f32, tag=f"ci{l}")
                cout = dp.tile([128, 8], f32, tag=f"co{l}")
                nc.sync.dma_start(cin[:], st[:])
                nc.gpsimd.collective_compute(
                    "AllReduce", OP.add, replica_groups=groups,
                    ins=[cin[:].opt()], outs=[cout[:].opt()])
                stg = sp_.tile([128, 8], f32, tag="sg")
                nc.sync.dma_start(stg[:], cout[:])

                # BN1: mu1/var1 from s1,s2 ; gsc = gc * rsqrt(var1+eps)
                mu1 = sp_.tile([128, 1], f32, tag="mu1")
                nc.vector.tensor_scalar_mul(out=mu1[:], in0=stg[:, 0:1],
                                            scalar1=NTOT_INV)
                v1 = sp_.tile([128, 1], f32, tag="v1")
                nc.vector.tensor_mul(out=v1[:], in0=mu1[:], in1=mu1[:])
                nc.vector.scalar_tensor_tensor(
                    out=v1[:], in0=stg[:, 1:2], scalar=NTOT_INV, in1=v1[:],
                    op0=OP.mult, op1=OP.subtract)
                r1 = rsqrt1(v1, "1")
                gsc = sp_.tile([128, 1], f32, tag="gsc")
                nc.vector.tensor_mul(out=gsc[:], in0=gc_sb[l][:], in1=r1[:])

                # BN2 stats derived: sum_xmid = gsc*s1 + s4,
                # sumsq_xmid = gsc^2*s2 + 2*gsc*s3 + s5
                sm2 = sp_.tile([128, 1], f32, tag="sm2")
                nc.vector.scalar_tensor_tensor(
                    out=sm2[:], in0=stg[:, 0:1], scalar=gsc[:], in1=stg[:, 3:4],
                    op0=OP.mult, op1=OP.add)
                # sq2 = gsc*(gsc*s2 + 2*s3) + s5
                sq2 = sp_.tile([128, 1], f32, tag="sq2")
                t1 = sp_.tile([128, 1], f32, tag="t1")
                nc.vector.tensor_scalar_mul(out=t1[:], in0=stg[:, 2:3], scalar1=2.0)
                nc.vector.scalar_tensor_tensor(
                    out=sq2[:], in0=stg[:, 1:2], scalar=gsc[:], in1=t1[:],
                    op0=OP.mult, op1=OP.add)
                nc.vector.scalar_tensor_tensor(
                    out=sq2[:], in0=sq2[:], scalar=gsc[:], in1=stg[:, 4:5],
                    op0=OP.mult, op1=OP.add)
                mu2 = sp_.tile([128, 1], f32, tag="mu2")
                nc.vector.tensor_scalar_mul(out=mu2[:], in0=sm2[:],
                                            scalar1=NTOT_INV)
                v2 = sp_.tile([128, 1], f32, tag="v2")
                nc.vector.tensor_mul(out=v2[:], in0=mu2[:], in1=mu2[:])
                nc.vector.scalar_tensor_tensor(
                    out=v2[:], in0=sq2[:], scalar=NTOT_INV, in1=v2[:],
                    op0=OP.mult, op1=OP.subtract)
                r2 = rsqrt1(v2, "2")
                sc2 = sp_.tile([128, 1], f32, tag="sc2")
                nc.vector.tensor_mul(out=sc2[:], in0=gn_sb[l][:], in1=r2[:])
                b2t = sp_.tile([128, 1], f32, tag="b2t")
                nc.vector.tensor_mul(out=b2t[:], in0=sc2[:], in1=mu2[:])
                nc.vector.tensor_sub(out=b2t[:], in0=bnb_sb[l][:], in1=b2t[:])

                # x_mid = gsc*agg + x (in place into agg; BN1 shift cancels
                # in BN2), then x_new = relu(sc2*x_mid + b2t)
                nc.vector.scalar_tensor_tensor(
                    out=agg[:], in0=agg[:], scalar=gsc[:], in1=xT_f[:],
                    op0=OP.mult, op1=OP.add)
                xT_f = xp.tile([128, NN], f32, tag="xf32")
                xT_bf = xb.tile([128, NN], bf16, tag="xbf")
                if l < L - 1:
                    stripes = [xb.tile([128, 1024], bf16, tag=f"str{g4}", name=f"strt{g4}")
                               for g4 in range(GPG)]
                # per-graph tail so layer l+1's first gathers start while
                # later graphs are still being transposed
                for g4 in range(GPG):
                    sl = slice(g4 * 1024, (g4 + 1) * 1024)
                    nc.scalar.activation(out=xT_f[:, sl], in_=agg[:, sl],
                                         func=AF.Relu, bias=b2t[:],
                                         scale=sc2[:])
                    nc.vector.tensor_copy(out=xT_bf[:, sl], in_=xT_f[:, sl])
                    if l < L - 1:
                        make_stripes_graph(xT_bf, stripes[g4], g4)

            if debug:
                nc.sync.dma_start(xdbg_d[:], xT_f[:])
                nc.sync.dma_start(adbg_d[:], agg[:])

            # ---------------- readout ----------------
            gsum = sp_.tile([128, cfg.GP], f32, tag="gsum")
            nc.vector.tensor_reduce(
                out=gsum[:], in_=xT_f[:].rearrange("p (g s) -> p g s", s=cfg.S),
                axis=mybir.AxisListType.X, op=OP.add)
            ph = ppt.tile([128, cfg.GP], f32, tag="tp")
            nc.tensor.matmul(ph[:], w1_sb[:], gsum[:], start=True, stop=True)
            h = sp_.tile([128, cfg.GP], f32, tag="h")
            nc.scalar.activation(out=h[:], in_=ph[:], func=AF.Relu,
                                 bias=b1_sb[:], scale=1.0)
            po = ppt.tile([1, cfg.GP], f32, tag="tp2")
            nc.tensor.matmul(po[:], w2_sb[:], h[:], start=True, stop=True)
            osb = sp_.tile([1, cfg.GP], f32, tag="osb")
            nc.scalar.activation(out=osb[:], in_=po[:], func=AF.Identity,
                                 bias=b2_sb[:], scale=1.0)
            nc.sync.dma_start(out_d[:], osb[:])

    nc.compile()
    return nc


def prep_inputs(inputs, cfg):
    """Full inputs -> per-core input maps (host-side sharding + layout)."""
    bfc = lambda a: np.asarray(a, np.float32).astype(ml_dtypes.bfloat16)
    z = np.asarray(inputs["z"])
    pos = np.asarray(inputs["pos"], np.float32)
    ei = np.asarray(inputs["edge_index"])
    row, col = ei[0].astype(np.int64), ei[1].astype(np.int64)
    Wf = np.asarray(inputs["Wf"], np.float32)
    Ws = np.asarray(inputs["Ws"], np.float32)
    bf_ = np.asarray(inputs["bf"], np.float32)
    bs_ = np.asarray(inputs["bs"], np.float32)
    gc = np.asarray(inputs["gc"], np.float32)
    gn = np.asarray(inputs["gn"], np.float32)
    bnb = np.asarray(inputs["bn_b"], np.float32)
    W1 = np.asarray(inputs["W1"], np.float32)
    b1 = np.asarray(inputs["b1"], np.float32)
    W2 = np.asarray(inputs["W2"], np.float32)
    b2 = np.asarray(inputs["b2"], np.float32)
    emb = np.asarray(inputs["emb"], np.float32)

    D, L = cfg.D, cfg.L
    # lhsT for the one-hot matmul: out[f, n] = sum_p emb_t[p, f] * oh[p, n]
    emb_t = np.zeros((128, 128), np.float32)
    emb_t[:emb.shape[0], :] = emb

    wcol = np.stack([np.stack([bfc(Wf[l, :D]), bfc(Ws[l, :D])]) for l in range(L)])
    wrow = np.stack([np.stack([bfc(Wf[l, D:2 * D]), bfc(Ws[l, D:2 * D])])
                     for l in range(L)])
    wdst = np.stack([np.stack([bfc(Wf[l, 2 * D:2 * D + 1]),
                               bfc(Ws[l, 2 * D:2 * D + 1])]) for l in range(L)])
    biases = np.stack([np.stack([bf_[l], bs_[l]]) for l in range(L)])

    dist_full = np.sqrt(
        ((pos[row] - pos[col]) ** 2).sum(-1)).astype(np.float32)  # [E]

    brow = np.stack([bfc(emb_t @ Wf[0, D:2 * D]), bfc(emb_t @ Ws[0, D:2 * D])])
    shared = dict(
        emb_t=emb_t, wcol=wcol, wrow=wrow, wdst=wdst, bias=biases,
        gc_p=gc, gn_p=gn, bnb_p=bnb,
        w1_p=(W1 / cfg.S).astype(np.float32),
        b1_p=b1, w2_p=W2, b2_p=b2.reshape(1, 1),
        ident=np.eye(128, dtype=np.float32).astype(ml_dtypes.bfloat16),
        brow=brow,
        iota_p=np.arange(128, dtype=np.float32).reshape(128, 1),
        ones_r=np.ones((1, 128), np.float32).astype(ml_dtypes.bfloat16),
    )

    maps = []
    for c in range(cfg.NCORE):
        n0, n1 = c * cfg.NN, (c + 1) * cfg.NN
        e0, e1 = c * cfg.NE, (c + 1) * cfg.NE
        zc = z[n0:n1]
        rl = row[e0:e1] - n0
        assert rl.min() >= 0 and rl.max() < cfg.NN, "edges cross core boundary"
        # graph-local indices (gather sources are per-graph stripe tiles)
        rl = rl - (np.arange(cfg.NE) // (cfg.S * cfg.DEG)) * cfg.S
        assert rl.min() >= 0 and rl.max() < cfg.S, "edges cross graph boundary"
        oh = (zc[None, :] == np.arange(128)[:, None])
        m = dict(shared)
        m.update(
            oh_z=oh.astype(np.float32),
            ridx=wrap16(rl),
            dist_p=bfc(dist_full[e0:e1].reshape(128, cfg.NE // 128)),
            zrow_p=bfc(z[row[e0:e1]].reshape(128, cfg.NE // 128)),
        )
        maps.append(m)
    return maps


_CACHE = {}


def make_runner(nc, n_cores):
    """Build a reusable jitted PJRT executable for `nc` (one NEFF compile +
    load; repeat calls only transfer inputs and execute)."""
    import jax
    from jax.sharding import Mesh, PartitionSpec
    from jax.experimental.shard_map import shard_map
    from concourse.bass2jax import (_bass_exec_p, install_neuronx_cc_hook,
                                    partition_id_tensor)
    import concourse.mybir as mybir

    install_neuronx_cc_hook()
    partition_name = (nc.partition_id_tensor.name
                      if nc.partition_id_tensor else None)
    in_names, out_names, out_avals, zero_outs = [], [], [], []
    for alloc in nc.m.functions[0].allocations:
        if not isinstance(alloc, mybir.MemoryLocationSet):
            continue
        name = alloc.memorylocations[0].name
        if alloc.kind == "ExternalInput":
            if name != partition_name:
                in_names.append(name)
        elif alloc.kind == "ExternalOutput":
            shape = tuple(alloc.tensor_shape)
            dtype = mybir.dt.np(alloc.dtype)
            out_names.append(name)
            out_avals.append(jax.core.ShapedArray(shape, dtype))
            zero_outs.append(np.zeros(shape, dtype))
    n_params = len(in_names)
    n_outs = len(out_avals)
    all_in_names = list(in_names) + list(out_names)
    if partition_name is not None:
        all_in_names.append(partition_name)
    donate = tuple(range(n_params, n_params + n_outs))

    def _body(*args):
        operands = list(args)
        if partition_name is not None:
            operands.append(partition_id_tensor())
        outs = _bass_exec_p.bind(
            *operands, out_avals=tuple(out_avals),
            in_names=tuple(all_in_names), out_names=tuple(out_names),
            lowering_input_output_aliases=(), sim_require_finite=True,
            sim_require_nnan=True, nc=nc)
        return tuple(outs)

    devices = jax.devices()[:n_cores]
    mesh = Mesh(np.asarray(devices), ("core",))
    in_specs = (PartitionSpec("core"),) * (n_params + n_outs)
    out_specs = (PartitionSpec("core"),) * n_outs
    sharded = jax.jit(
        shard_map(_body, mesh=mesh, in_specs=in_specs, out_specs=out_specs,
                  check_rep=False),
        donate_argnums=donate, keep_unused=True)

    def run(maps, device_inputs=None):
        if device_inputs is None:
            device_inputs = stage(maps)
        concat_zeros = [
            np.zeros((n_cores * z.shape[0], *z.shape[1:]), z.dtype)
            for z in zero_outs]
        out_arrs = sharded(*device_inputs, *concat_zeros)
        return [
            {name: np.asarray(out_arrs[i]).reshape(n_cores, *out_avals[i].shape)[c]
             for i, name in enumerate(out_names)}
            for c in range(n_cores)]

    def stage(maps):
        from jax.sharding import NamedSharding
        sh = NamedSharding(mesh, PartitionSpec("core"))
        return [
            jax.device_put(
                np.concatenate([np.asarray(maps[c][nm])
                                for c in range(n_cores)], axis=0), sh)
            for nm in in_names]

    run.stage = stage
    return run


def _get_nc(cfg_key=()):
    if cfg_key not in _CACHE:
        cfg = Cfg()
        nc = build_nc(cfg)
        runner = make_runner(nc, cfg.NCORE)
        _CACHE[cfg_key] = (cfg, nc, runner)
    return _CACHE[cfg_key]


def kernel(**inputs):
    cfg, nc, runner = _get_nc()
    # structural preconditions from the generator: edges grouped by target,
    # exactly DEG edges per node; batch = repeat(arange(G), S)
    ei = np.asarray(inputs["edge_index"])
    N = cfg.NN * cfg.NCORE
    assert np.array_equal(ei[1], np.repeat(np.arange(N), cfg.DEG)), \
        "edge_index[1] must be repeat(arange(N), DEG)"
    maps = prep_inputs(inputs, cfg)
    results = runner(maps)
    outs = [results[c]["out4"].reshape(-1) for c in range(cfg.NCORE)]
    return np.concatenate(outs).astype(np.float32)


if __name__ == "__main__":
    cfg = Cfg()
    nc = build_nc(cfg)
    print("built + compiled OK")


# revision 59
# speedup vs baseline: 1.0794x; 1.0388x over previous
"""CGCNN-style GNN message passing on 8 Trainium2 NeuronCores.

Sharding: data-parallel over graphs (4 graphs / core).  Each core holds its
4096 nodes and their 131072 in-edges entirely locally (edges never cross
graph boundaries).  Only BatchNorm batch statistics are all-reduced (one
[128,8] f32 AllReduce per layer: {sum agg, sum agg^2, sum agg*x, sum x,
sum x^2} lets both BN1 and BN2 affines be derived from a single exchange).

Device-side layout is feature-major ([128 features x nodes/edges]):
  - x kept feature-major (f32 + bf16 copies) and as per-graph node-major
    "stripes" (gather source: node m -> partition m%128, 256B rank m//128).
  - per-edge pre-activations a = W_col^T x[col] + w_d*dist + W_row^T x[row]
    accumulate on the PE into PSUM, weight-major per psum tile (each
    LDWEIGHTS feeds two back-to-back 512-col matmuls):
      * W_col term: rhs = broadcast AP view of x (col = edge//32)
      * dist term:  K=1 matmul, rhs = [1, 512] dist slices DMAd from DRAM
      * W_row term: rhs = gathered x columns (dma_gather transpose=True,
        all on SWDGE queue 0 -- concurrent desc-gen on multiple queues
        corrupts data on this ucode build, hardware-verified)
  - layer 0 needs no gathers: x0[row] = emb[z[row]] and z < 128, so the
    row term is brow^T onehot(z[row]) with the one-hot built on device
    (K=1 replication matmul + DVE is_equal against the partition index).
  - activations batched per gate to avoid ACT table thrash: all sigmoid
    tiles of a chunk, then all exp tiles, then one Ln(1+u) pass.
  - message product on DVE (bf16 2x), 32-edge segment sum via tensor_reduce.
  - x0 = emb[z] via one-hot matmul (host ships the [128, NN] one-hot of z).
"""

import os
import sys

sys.path.insert(0, "/opt/trn_rl_repo")

import numpy as np
import ml_dtypes

import concourse.bass as bass
import concourse.bacc as bacc
import concourse.mybir as mybir
import concourse.tile as tile

f32 = mybir.dt.float32
bf16 = mybir.dt.bfloat16
i16 = mybir.dt.int16
AF = mybir.ActivationFunctionType
OP = mybir.AluOpType

EPS = 1e-5


class Cfg:
    def __init__(self, G=32, S=1024, DEG=32, D=128, L=4, NCORE=8, CH=8192):
        self.G, self.S, self.DEG, self.D, self.L, self.NCORE = G, S, DEG, D, L, NCORE
        self.GP = G // NCORE            # graphs per core
        self.NN = self.GP * S           # nodes per core
        self.NE = self.NN * DEG         # edges per core
        self.CH = min(CH, self.NE)      # edge chunk
        self.NCH = self.NE // self.CH
        self.PT = 1024                  # psum tile cols (2 banks)
        self.NT = 512                   # matmul N per region (1 bank)
        assert self.NN % 128 == 0 and self.NE % self.CH == 0
        assert self.CH % self.PT == 0 and self.PT % self.NT == 0
        assert self.NE // self.PT == 128  # dist partition-major layout
        self.NTOT = self.NN * NCORE     # total nodes (BN denominator)


def wrap16(idx):
    """[n] -> [128, n/16] int16 wrapped layout for dma_gather index tensors."""
    a = np.asarray(idx, np.int16).reshape(-1, 16).T  # [16, n/16]
    return np.tile(a, (8, 1)).copy()                 # [128, n/16]


def build_nc(cfg, debug=False):
    NN, NE, D, L, CH = cfg.NN, cfg.NE, cfg.D, cfg.L, cfg.CH
    NT, PT, DEG, NCH = cfg.NT, cfg.PT, cfg.DEG, cfg.NCH
    TPC = CH // PT                       # psum tiles per chunk

    nc = bacc.Bacc("TRN2", target_bir_lowering=False, debug=False,
                   num_devices=cfg.NCORE, num_swdge_queues=4)

    def din(name, shape, dt):
        return nc.dram_tensor(name, shape, dt, kind="ExternalInput")

    emb_d = din("emb_t", [128, 128], f32)                  # emb padded [z, f]
    oh_d = din("oh_z", [128, NN], f32)                     # one-hot of z
    ridx_d = din("ridx", [128, NE // 16], i16)
    dist_d = din("dist_p", [128, NE // 128], bf16)         # row=global tile idx
    zrow_d = din("zrow_p", [128, NE // 128], bf16)         # z[row], same layout
    brow_d = din("brow", [2, 128, 128], bf16)              # emb_pad @ Wrow[0,g]
    iota_d = din("iota_p", [128, 1], f32)
    ones_d = din("ones_r", [1, 128], bf16)
    wcol_d = din("wcol", [L, 2, 128, 128], bf16)
    wrow_d = din("wrow", [L, 2, 128, 128], bf16)
    wdst_d = din("wdst", [L, 2, 1, 128], bf16)
    bias_d = din("bias", [L, 2, 128], f32)
    gc_d = din("gc_p", [L, 128], f32)
    gn_d = din("gn_p", [L, 128], f32)
    bnb_d = din("bnb_p", [L, 128], f32)
    w1_d = din("w1_p", [128, 128], f32)                    # pre-scaled by 1/S
    b1_d = din("b1_p", [128], f32)
    w2_d = din("w2_p", [128, 1], f32)
    b2_d = din("b2_p", [1, 1], f32)
    ident_d = din("ident", [128, 128], bf16)

    out_d = nc.dram_tensor("out4", [1, cfg.GP], f32, kind="ExternalOutput")
    if debug:
        xdbg_d = nc.dram_tensor("xdbg", [128, NN], f32, kind="ExternalOutput")
        adbg_d = nc.dram_tensor("adbg", [128, NN], f32, kind="ExternalOutput")

    groups = [list(range(cfg.NCORE))]
    NTOT_INV = 1.0 / float(cfg.NTOT)
    # Concurrent dma_gathers (multiple SWDGE queues) corrupt data on this
    # ucode build (hardware-verified: overlapping desc-gen shifts output
    # columns nondeterministically) -- keep every gather on queue 0, serial.
    QORDER = tuple(int(x) for x in os.environ.get("KQORDER", "0,0,0,0").split(","))

    with tile.TileContext(nc) as tc:
        with (
            tc.tile_pool(name="const", bufs=1) as cp,
            tc.tile_pool(name="xpool", bufs=2) as xp,
            tc.tile_pool(name="xbf", bufs=1) as xb,
            tc.tile_pool(name="node", bufs=1) as npo,
            tc.tile_pool(name="idxp", bufs=2) as ip,
            tc.tile_pool(name="distp", bufs=1) as dip,
            tc.tile_pool(name="gath", bufs=1) as gp,
            tc.tile_pool(name="acts", bufs=1) as ap_,
            tc.tile_pool(name="small", bufs=1) as sp_,
            tc.tile_pool(name="ps", bufs=3, space="PSUM") as pp,
            tc.tile_pool(name="pst", bufs=1, space="PSUM") as ppt,
            tc.tile_pool(name="dram", bufs=2, space="DRAM") as dp,
        ):
            # ---------------- constants ----------------
            emb_sb = cp.tile([128, 128], f32)
            nc.sync.dma_start(emb_sb[:], emb_d[:])
            ident_sb = cp.tile([128, 128], bf16)
            nc.sync.dma_start(ident_sb[:], ident_d[:])
            brow_sb = {}
            for g in range(2):
                t = cp.tile([128, 128], bf16, tag=f"br{g}")
                nc.sync.dma_start(t[:], brow_d[g])
                brow_sb[g] = t
            iota_sb = cp.tile([128, 1], f32)
            nc.sync.dma_start(iota_sb[:], iota_d[:])
            ones_sb = cp.tile([1, 128], bf16)
            nc.sync.dma_start(ones_sb[:], ones_d[:])
            wcol_sb, wrow_sb, wdst_sb, bias_sb = {}, {}, {}, {}
            for l in range(L):
                for g in range(2):
                    t = cp.tile([128, 128], bf16, tag=f"wc{l}{g}")
                    nc.sync.dma_start(t[:], wcol_d[l, g])
                    wcol_sb[l, g] = t
                    t = cp.tile([128, 128], bf16, tag=f"wr{l}{g}")
                    nc.sync.dma_start(t[:], wrow_d[l, g])
                    wrow_sb[l, g] = t
                    t = cp.tile([1, 128], bf16, tag=f"wd{l}{g}")
                    nc.sync.dma_start(t[:], wdst_d[l, g])
                    wdst_sb[l, g] = t
                    t = cp.tile([128, 1], f32, tag=f"bi{l}{g}")
                    nc.sync.dma_start(t[:], bias_d[l, g].rearrange("(p o) -> p o", o=1))
                    bias_sb[l, g] = t
            gc_sb, gn_sb, bnb_sb = {}, {}, {}
            for l in range(L):
                for nm, d_, dst in (("gc", gc_d, gc_sb), ("gn", gn_d, gn_sb),
                                    ("bb", bnb_d, bnb_sb)):
                    t = cp.tile([128, 1], f32, tag=f"{nm}{l}")
                    nc.sync.dma_start(t[:], d_[l].rearrange("(p o) -> p o", o=1))
                    dst[l] = t
            w1_sb = cp.tile([128, 128], f32)
            nc.sync.dma_start(w1_sb[:], w1_d[:])
            b1_sb = cp.tile([128, 1], f32)
            nc.sync.dma_start(b1_sb[:], b1_d[:].rearrange("(p o) -> p o", o=1))
            w2_sb = cp.tile([128, 1], f32)
            nc.sync.dma_start(w2_sb[:], w2_d[:])
            b2_sb = cp.tile([1, 1], f32)
            nc.sync.dma_start(b2_sb[:], b2_d[:])

            # ---------------- x0 = emb[z] via one-hot matmul ----------------

            def make_stripes_graph(src_bf, st, g4):
                """Graph g4's [128 f, 1024] block -> node-major stripes
                (node m -> partition m%128, byte range [(m//128)*256, +256)).
                4 transposes batched per PSUM tile, one copy each."""
                for j in range(2):
                    ptile = ppt.tile([128, 512], bf16, tag="tp")
                    for k in range(4):
                        r = j * 4 + k
                        nc.tensor.transpose(
                            ptile[:, k * 128:(k + 1) * 128],
                            src_bf[:, g4 * 1024 + r * 128:g4 * 1024 + (r + 1) * 128],
                            ident_sb[:])
                    nc.vector.tensor_copy(out=st[:, j * 512:(j + 1) * 512],
                                          in_=ptile[:])

            xT_f = xp.tile([128, NN], f32, tag="xf32")
            xT_bf = xb.tile([128, NN], bf16, tag="xbf")
            for t in range(NN // PT):
                ohc = npo.tile([128, PT], f32, tag=f"ohc{t % 2}")
                nc.sync.dma_start(ohc[:], oh_d[:, t * PT:(t + 1) * PT])
                ps0 = pp.tile([128, PT], f32, tag="ps")
                for u in range(PT // NT):
                    ou = slice(u * NT, (u + 1) * NT)
                    nc.tensor.matmul(ps0[:, ou], emb_sb[:], ohc[:, ou],
                                     start=True, stop=True)
                oc = slice(t * PT, (t + 1) * PT)
                nc.scalar.activation(out=xT_f[:, oc], in_=ps0[:],
                                     func=AF.Identity, bias=0.0, scale=1.0)
                nc.vector.tensor_copy(out=xT_bf[:, oc], in_=xT_f[:, oc])
            stripes = None   # layer 0 needs no gathers (one-hot z path)
            GPG = cfg.GP     # graphs per core (stripes are per-graph)

            def rsqrt1(v, tagp):
                """[128,1] var -> 1/sqrt(var+eps) with one Newton step."""
                nc.vector.tensor_scalar_add(out=v[:], in0=v[:], scalar1=EPS)
                s = sp_.tile([128, 1], f32, tag=f"s{tagp}")
                nc.scalar.sqrt(out=s[:], in_=v[:])
                r = sp_.tile([128, 1], f32, tag=f"r{tagp}")
                nc.vector.reciprocal(out=r[:], in_=s[:])
                a = sp_.tile([128, 1], f32, tag=f"a{tagp}")
                nc.vector.tensor_mul(out=a[:], in0=r[:], in1=r[:])
                nc.vector.tensor_mul(out=a[:], in0=v[:], in1=a[:])
                nc.vector.tensor_scalar(out=a[:], in0=a[:], scalar1=-0.5,
                                        scalar2=1.5, op0=OP.mult, op1=OP.add)
                nc.vector.tensor_mul(out=r[:], in0=r[:], in1=a[:])
                return r

            # ---------------- layers ----------------
            for l in range(L):
                agg = npo.tile([128, NN], f32, tag="agg")
                # node-level col terms A_c[g] = W_col^T x  ([128, NN] bf16):
                # the per-edge col contribution is constant within each
                # 32-edge group, so it is added into PSUM by one DVE op per
                # tile (broadcast view) instead of a K=128 matmul per region.
                ac_sb = {}
                for g in range(2):
                    act_ = xb.tile([128, NN], bf16, tag=f"ac{g}", name=f"ac{g}")
                    for t4 in range(NN // PT):
                        psA = pp.tile([128, PT], f32, tag="ps")
                        for u in range(PT // NT):
                            o = slice(t4 * PT + u * NT, t4 * PT + (u + 1) * NT)
                            nc.tensor.matmul(
                                psA[:, u * NT:(u + 1) * NT],
                                wcol_sb[l, g][:], xT_bf[:, o],
                                start=True, stop=True)
                        nc.vector.tensor_copy(
                            out=act_[:, t4 * PT:(t4 + 1) * PT], in_=psA[:])
                    ac_sb[g] = act_
                for c in range(NCH):
                    e0c = c * CH
                    if l > 0:
                        idxc = ip.tile([128, CH // 16], i16, tag=f"idx{c % 2}")
                        nc.sync.dma_start(
                            idxc[:], ridx_d[:, e0c // 16:(e0c + CH) // 16])
                        xg = gp.tile([128, CH], bf16, tag=f"xg{c % 2}")
                        # two half-gathers: the first half's tiles can start
                        # on the PE while the second half's descriptors are
                        # still being generated / drained
                        for h in range(2):
                            hs = slice(h * CH // 2, (h + 1) * CH // 2)
                            nc.gpsimd.dma_gather(
                                out_ap=xg[:, hs].rearrange(
                                    "p (a n) -> p a n", a=1),
                                in_ap=stripes[c // 4][:],
                                idxs_ap=idxc[:, h * CH // 32:(h + 1) * CH // 32],
                                num_idxs=CH // 2, num_idxs_reg=CH // 2,
                                elem_size=128,
                                transpose=True, sbuf_tokens_per_rank=128,
                                sbuf_free_dim_per_rank=256,
                                sbuf_free_dim_pad_per_rank=0, sbuf_byte_offset=0,
                                single_packet=False, queue_num=QORDER[c % 4])
                    sgf = ap_.tile([128, CH], bf16, tag="sgf")
                    usb = ap_.tile([128, CH], bf16, tag="usb")
                    if l == 0:
                        # layer-0 x0[row] term: one-hot of z[row] (values<128)
                        # built on device -- replicate zrow across partitions
                        # on the (otherwise idle) GPSIMD daisy chain, compare
                        # to the partition index on DVE (bf16 4x mode).
                        xg = gp.tile([128, CH], bf16, tag=f"xg{c % 2}")
                        zrep = gp.tile([128, CH], bf16, tag="zrep")
                        for t in range(TPC):
                            q = c * TPC + t
                            zr = dip.tile([1, PT], bf16, tag=f"zr{t % 2}")
                            nc.sync.dma_start(zr[:], zrow_d[q:q + 1, :])
                            nc.gpsimd.partition_broadcast(
                                zrep[:, t * PT:(t + 1) * PT], zr[0:1, :])
                            nc.vector.tensor_scalar(
                                out=xg[:, t * PT:(t + 1) * PT],
                                in0=zrep[:, t * PT:(t + 1) * PT],
                                scalar1=iota_sb[:], scalar2=None,
                                op0=OP.is_equal)
                            # z < 100, so one-hot row 100 is free: carry dist
                            # there and fold the wdst term into the brow
                            # matmul (brow row 100 = wdst, set host-side)
                            nc.sync.dma_start(
                                xg[100:101, t * PT:(t + 1) * PT],
                                dist_d[q:q + 1, :])
                    # gate-major: all f tiles (sigmoid set), then all s tiles
                    # (exp), then one Ln(1+u) pass -> 2 table loads per chunk.
                    dists = {}
                    if l > 0:
                        for t in range(TPC):
                            q = c * TPC + t          # global psum-tile index
                            dc = dip.tile([1, PT], bf16, tag=f"dc{t}")
                            nc.sync.dma_start(dc[:], dist_d[q:q + 1, :])
                            dists[t] = dc
                    for g in range(2):
                        for t in range(TPC):
                            ps = pp.tile([128, PT], f32, tag="ps")
                            # weight-major over the two 512-regions so each
                            # LDWEIGHTS serves two back-to-back matmuls
                            for wi in range(2):
                                if l == 0 and wi == 0:
                                    continue  # dist folded into brow row 100
                                for u in range(PT // NT):
                                    ecl = t * PT + u * NT
                                    o = slice(ecl, ecl + NT)
                                    ou = slice(u * NT, (u + 1) * NT)
                                    if wi == 0:
                                        w = wdst_sb[l, g]
                                        rhs = dists[t][0:1, ou]
                                    else:
                                        w = (brow_sb[g] if l == 0
                                             else wrow_sb[l, g])
                                        rhs = xg[:, o]
                                    nc.tensor.matmul(
                                        ps[:, ou], w[:], rhs,
                                        start=(wi == 0 or l == 0),
                                        stop=(wi == 1))
                            # + col term (constant per 32-edge group)
                            n0 = (e0c + t * PT) // DEG
                            nn_ = PT // DEG
                            nc.vector.tensor_add(
                                out=ps[:].rearrange("p (n k) -> p n k", k=DEG),
                                in0=ps[:].rearrange("p (n k) -> p n k", k=DEG),
                                in1=(ac_sb[g][:, n0:n0 + nn_]
                                     .unsqueeze(2)
                                     .to_broadcast((128, nn_, DEG))))
                            oc = slice(t * PT, (t + 1) * PT)
                            if g == 0:
                                nc.scalar.activation(
                                    out=sgf[:, oc], in_=ps[:], func=AF.Sigmoid,
                                    bias=bias_sb[l, 0][:], scale=1.0)
                            else:
                                nc.scalar.activation(
                                    out=usb[:, oc], in_=ps[:], func=AF.Exp,
                                    bias=bias_sb[l, 1][:], scale=1.0)
                    # softplus tail: sp = ln(1 + u)   (in place)
                    nc.scalar.activation(out=usb[:], in_=usb[:], func=AF.Ln,
                                         bias=1.0, scale=1.0)
                    # message product (in place into sgf)
                    nc.vector.tensor_mul(out=sgf[:], in0=sgf[:], in1=usb[:])
                    # segment sum over DEG=32
                    nc.vector.tensor_reduce(
                        out=agg[:, e0c // DEG:(e0c + CH) // DEG],
                        in_=sgf[:].rearrange("p (n k) -> p n k", k=DEG),
                        axis=mybir.AxisListType.X, op=OP.add)

                # ---- BN stats: one AllReduce of [sum agg, sum agg^2,
                #      sum agg*x, sum x, sum x^2] ----
                st = sp_.tile([128, 8], f32, tag="st")
                nc.vector.tensor_reduce(out=st[:, 0:1], in_=agg[:],
                                        axis=mybir.AxisListType.X, op=OP.add)
                nc.vector.tensor_reduce(out=st[:, 3:4], in_=xT_f[:],
                                        axis=mybir.AxisListType.X, op=OP.add)
                NP4 = 4
                NQ = NN // NP4
                pq = sp_.tile([128, 3 * NP4], f32, tag="pq")
                scr = npo.tile([128, NQ], f32, tag="scratch")
                for q in range(NP4):
                    qs = slice(q * NQ, (q + 1) * NQ)
                    nc.vector.scalar_tensor_tensor(
                        out=scr[:], in0=agg[:, qs], scalar=0.0, in1=agg[:, qs],
                        op0=OP.add, op1=OP.mult, accum_out=pq[:, q:q + 1])
                    nc.vector.scalar_tensor_tensor(
                        out=scr[:], in0=agg[:, qs], scalar=0.0, in1=xT_f[:, qs],
                        op0=OP.add, op1=OP.mult,
                        accum_out=pq[:, NP4 + q:NP4 + q + 1])
                    nc.vector.scalar_tensor_tensor(
                        out=scr[:], in0=xT_f[:, qs], scalar=0.0, in1=xT_f[:, qs],
                        op0=OP.add, op1=OP.mult,
                        accum_out=pq[:, 2 * NP4 + q:2 * NP4 + q + 1])
                for k, col in ((0, 1), (1, 2), (2, 4)):
                    nc.vector.tensor_reduce(
                        out=st[:, col:col + 1],
                        in_=pq[:, k * NP4:(k + 1) * NP4],
                        axis=mybir.AxisListType.X, op=OP.add)

                cin = dp.tile([128, 8], f32, tag=f"ci{l}")
                cout = dp.tile([128, 8], f32, tag=f"co{l}")
                nc.sync.dma_start(cin[:], st[:])
                nc.gpsimd.collective_compute(
                    "AllReduce", OP.add, replica_groups=groups,
                    ins=[cin[:].opt()], outs=[cout[:].opt()])
                stg = sp_.tile([128, 8], f32, tag="sg")
                nc.sync.dma_start(stg[:], cout[:])

                # BN1: mu1/var1 from s1,s2 ; gsc = gc * rsqrt(var1+eps)
                mu1 = sp_.tile([128, 1], f32, tag="mu1")
                nc.vector.tensor_scalar_mul(out=mu1[:], in0=stg[:, 0:1],
                                            scalar1=NTOT_INV)
                v1 = sp_.tile([128, 1], f32, tag="v1")
                nc.vector.tensor_mul(out=v1[:], in0=mu1[:], in1=mu1[:])
                nc.vector.scalar_tensor_tensor(
                    out=v1[:], in0=stg[:, 1:2], scalar=NTOT_INV, in1=v1[:],
                    op0=OP.mult, op1=OP.subtract)
                r1 = rsqrt1(v1, "1")
                gsc = sp_.tile([128, 1], f32, tag="gsc")
                nc.vector.tensor_mul(out=gsc[:], in0=gc_sb[l][:], in1=r1[:])

                # BN2 stats derived: sum_xmid = gsc*s1 + s4,
                # sumsq_xmid = gsc^2*s2 + 2*gsc*s3 + s5
                sm2 = sp_.tile([128, 1], f32, tag="sm2")
                nc.vector.scalar_tensor_tensor(
                    out=sm2[:], in0=stg[:, 0:1], scalar=gsc[:], in1=stg[:, 3:4],
                    op0=OP.mult, op1=OP.add)
                # sq2 = gsc*(gsc*s2 + 2*s3) + s5
                sq2 = sp_.tile([128, 1], f32, tag="sq2")
                t1 = sp_.tile([128, 1], f32, tag="t1")
                nc.vector.tensor_scalar_mul(out=t1[:], in0=stg[:, 2:3], scalar1=2.0)
                nc.vector.scalar_tensor_tensor(
                    out=sq2[:], in0=stg[:, 1:2], scalar=gsc[:], in1=t1[:],
                    op0=OP.mult, op1=OP.add)
                nc.vector.scalar_tensor_tensor(
                    out=sq2[:], in0=sq2[:], scalar=gsc[:], in1=stg[:, 4:5],
                    op0=OP.mult, op1=OP.add)
                mu2 = sp_.tile([128, 1], f32, tag="mu2")
                nc.vector.tensor_scalar_mul(out=mu2[:], in0=sm2[:],
                                            scalar1=NTOT_INV)
                v2 = sp_.tile([128, 1], f32, tag="v2")
                nc.vector.tensor_mul(out=v2[:], in0=mu2[:], in1=mu2[:])
                nc.vector.scalar_tensor_tensor(
                    out=v2[:], in0=sq2[:], scalar=NTOT_INV, in1=v2[:],
                    op0=OP.mult, op1=OP.subtract)
                r2 = rsqrt1(v2, "2")
                sc2 = sp_.tile([128, 1], f32, tag="sc2")
                nc.vector.tensor_mul(out=sc2[:], in0=gn_sb[l][:], in1=r2[:])
                b2t = sp_.tile([128, 1], f32, tag="b2t")
                nc.vector.tensor_mul(out=b2t[:], in0=sc2[:], in1=mu2[:])
                nc.vector.tensor_sub(out=b2t[:], in0=bnb_sb[l][:], in1=b2t[:])

                # x_mid = gsc*agg + x (in place into agg; BN1 shift cancels
                # in BN2), then x_new = relu(sc2*x_mid + b2t)
                nc.vector.scalar_tensor_tensor(
                    out=agg[:], in0=agg[:], scalar=gsc[:], in1=xT_f[:],
                    op0=OP.mult, op1=OP.add)
                xT_f = xp.tile([128, NN], f32, tag="xf32")
                xT_bf = xb.tile([128, NN], bf16, tag="xbf")
                if l < L - 1:
                    stripes = [xb.tile([128, 1024], bf16, tag=f"str{g4}", name=f"strt{g4}")
                               for g4 in range(GPG)]
                # per-graph tail so layer l+1's first gathers start while
                # later graphs are still being transposed
                for g4 in range(GPG):
                    sl = slice(g4 * 1024, (g4 + 1) * 1024)
                    nc.scalar.activation(out=xT_f[:, sl], in_=agg[:, sl],
                                         func=AF.Relu, bias=b2t[:],
                                         scale=sc2[:])
                    nc.vector.tensor_copy(out=xT_bf[:, sl], in_=xT_f[:, sl])
                    if l < L - 1:
                        make_stripes_graph(xT_bf, stripes[g4], g4)

            if debug:
                nc.sync.dma_start(xdbg_d[:], xT_f[:])
                nc.sync.dma_start(adbg_d[:], agg[:])

            # ---------------- readout ----------------
            gsum = sp_.tile([128, cfg.GP], f32, tag="gsum")
            nc.vector.tensor_reduce(
                out=gsum[:], in_=xT_f[:].rearrange("p (g s) -> p g s", s=cfg.S),
                axis=mybir.AxisListType.X, op=OP.add)
            ph = ppt.tile([128, cfg.GP], f32, tag="tp")
            nc.tensor.matmul(ph[:], w1_sb[:], gsum[:], start=True, stop=True)
            h = sp_.tile([128, cfg.GP], f32, tag="h")
            nc.scalar.activation(out=h[:], in_=ph[:], func=AF.Relu,
                                 bias=b1_sb[:], scale=1.0)
            po = ppt.tile([1, cfg.GP], f32, tag="tp2")
            nc.tensor.matmul(po[:], w2_sb[:], h[:], start=True, stop=True)
            osb = sp_.tile([1, cfg.GP], f32, tag="osb")
            nc.scalar.activation(out=osb[:], in_=po[:], func=AF.Identity,
                                 bias=b2_sb[:], scale=1.0)
            nc.sync.dma_start(out_d[:], osb[:])

    nc.compile()
    return nc


def prep_inputs(inputs, cfg):
    """Full inputs -> per-core input maps (host-side sharding + layout)."""
    bfc = lambda a: np.asarray(a, np.float32).astype(ml_dtypes.bfloat16)
    z = np.asarray(inputs["z"])
    pos = np.asarray(inputs["pos"], np.float32)
    ei = np.asarray(inputs["edge_index"])
    row, col = ei[0].astype(np.int64), ei[1].astype(np.int64)
    Wf = np.asarray(inputs["Wf"], np.float32)
    Ws = np.asarray(inputs["Ws"], np.float32)
    bf_ = np.asarray(inputs["bf"], np.float32)
    bs_ = np.asarray(inputs["bs"], np.float32)
    gc = np.asarray(inputs["gc"], np.float32)
    gn = np.asarray(inputs["gn"], np.float32)
    bnb = np.asarray(inputs["bn_b"], np.float32)
    W1 = np.asarray(inputs["W1"], np.float32)
    b1 = np.asarray(inputs["b1"], np.float32)
    W2 = np.asarray(inputs["W2"], np.float32)
    b2 = np.asarray(inputs["b2"], np.float32)
    emb = np.asarray(inputs["emb"], np.float32)

    D, L = cfg.D, cfg.L
    # lhsT for the one-hot matmul: out[f, n] = sum_p emb_t[p, f] * oh[p, n]
    emb_t = np.zeros((128, 128), np.float32)
    emb_t[:emb.shape[0], :] = emb

    wcol = np.stack([np.stack([bfc(Wf[l, :D]), bfc(Ws[l, :D])]) for l in range(L)])
    wrow = np.stack([np.stack([bfc(Wf[l, D:2 * D]), bfc(Ws[l, D:2 * D])])
                     for l in range(L)])
    wdst = np.stack([np.stack([bfc(Wf[l, 2 * D:2 * D + 1]),
                               bfc(Ws[l, 2 * D:2 * D + 1])]) for l in range(L)])
    biases = np.stack([np.stack([bf_[l], bs_[l]]) for l in range(L)])

    dist_full = np.sqrt(
        ((pos[row] - pos[col]) ** 2).sum(-1)).astype(np.float32)  # [E]

    brow = np.stack([bfc(emb_t @ Wf[0, D:2 * D]), bfc(emb_t @ Ws[0, D:2 * D])])
    # z < 100 so one-hot rows 100+ are free: row 100 carries dist, its
    # weight row is wdst (folds the dist term into the layer-0 row matmul)
    brow[0, 100, :] = bfc(Wf[0, 2 * D])
    brow[1, 100, :] = bfc(Ws[0, 2 * D])
    shared = dict(
        emb_t=emb_t, wcol=wcol, wrow=wrow, wdst=wdst, bias=biases,
        gc_p=gc, gn_p=gn, bnb_p=bnb,
        w1_p=(W1 / cfg.S).astype(np.float32),
        b1_p=b1, w2_p=W2, b2_p=b2.reshape(1, 1),
        ident=np.eye(128, dtype=np.float32).astype(ml_dtypes.bfloat16),
        brow=brow,
        iota_p=np.arange(128, dtype=np.float32).reshape(128, 1),
        ones_r=np.ones((1, 128), np.float32).astype(ml_dtypes.bfloat16),
    )

    maps = []
    for c in range(cfg.NCORE):
        n0, n1 = c * cfg.NN, (c + 1) * cfg.NN
        e0, e1 = c * cfg.NE, (c + 1) * cfg.NE
        zc = z[n0:n1]
        rl = row[e0:e1] - n0
        assert rl.min() >= 0 and rl.max() < cfg.NN, "edges cross core boundary"
        # graph-local indices (gather sources are per-graph stripe tiles)
        rl = rl - (np.arange(cfg.NE) // (cfg.S * cfg.DEG)) * cfg.S
        assert rl.min() >= 0 and rl.max() < cfg.S, "edges cross graph boundary"
        oh = (zc[None, :] == np.arange(128)[:, None])
        m = dict(shared)
        m.update(
            oh_z=oh.astype(np.float32),
            ridx=wrap16(rl),
            dist_p=bfc(dist_full[e0:e1].reshape(128, cfg.NE // 128)),
            zrow_p=bfc(z[row[e0:e1]].reshape(128, cfg.NE // 128)),
        )
        maps.append(m)
    return maps


_CACHE = {}


def make_runner(nc, n_cores):
    """Build a reusable jitted PJRT executable for `nc` (one NEFF compile +
    load; repeat calls only transfer inputs and execute)."""
    import jax
    from jax.sharding import Mesh, PartitionSpec
    from jax.experimental.shard_map import shard_map
    from concourse.bass2jax import (_bass_exec_p, install_neuronx_cc_hook,
                                    partition_id_tensor)
    import concourse.mybir as mybir

    install_neuronx_cc_hook()
    partition_name = (nc.partition_id_tensor.name
                      if nc.partition_id_tensor else None)
    in_names, out_names, out_avals, zero_outs = [], [], [], []
    for alloc in nc.m.functions[0].allocations:
        if not isinstance(alloc, mybir.MemoryLocationSet):
            continue
        name = alloc.memorylocations[0].name
        if alloc.kind == "ExternalInput":
            if name != partition_name:
                in_names.append(name)
        elif alloc.kind == "ExternalOutput":
            shape = tuple(alloc.tensor_shape)
            dtype = mybir.dt.np(alloc.dtype)
            out_names.append(name)
            out_avals.append(jax.core.ShapedArray(shape, dtype))
            zero_outs.append(np.zeros(shape, dtype))
    n_params = len(in_names)
    n_outs = len(out_avals)
    all_in_names = list(in_names) + list(out_names)
    if partition_name is not None:
        all_in_names.append(partition_name)
    donate = tuple(range(n_params, n_params + n_outs))

    def _body(*args):
        operands = list(args)
        if partition_name is not None:
            operands.append(partition_id_tensor())
        outs = _bass_exec_p.bind(
            *operands, out_avals=tuple(out_avals),
            in_names=tuple(all_in_names), out_names=tuple(out_names),
            lowering_input_output_aliases=(), sim_require_finite=True,
            sim_require_nnan=True, nc=nc)
        return tuple(outs)

    devices = jax.devices()[:n_cores]
    mesh = Mesh(np.asarray(devices), ("core",))
    in_specs = (PartitionSpec("core"),) * (n_params + n_outs)
    out_specs = (PartitionSpec("core"),) * n_outs
    sharded = jax.jit(
        shard_map(_body, mesh=mesh, in_specs=in_specs, out_specs=out_specs,
                  check_rep=False),
        donate_argnums=donate, keep_unused=True)

    def run(maps, device_inputs=None):
        if device_inputs is None:
            device_inputs = stage(maps)
        concat_zeros = [
            np.zeros((n_cores * z.shape[0], *z.shape[1:]), z.dtype)
            for z in zero_outs]
        out_arrs = sharded(*device_inputs, *concat_zeros)
        return [
            {name: np.asarray(out_arrs[i]).reshape(n_cores, *out_avals[i].shape)[c]
             for i, name in enumerate(out_names)}
            for c in range(n_cores)]

    def stage(maps):
        from jax.sharding import NamedSharding
        sh = NamedSharding(mesh, PartitionSpec("core"))
        return [
            jax.device_put(
                np.concatenate([np.asarray(maps[c][nm])
                                for c in range(n_cores)], axis=0), sh)
            for nm in in_names]

    run.stage = stage
    return run


def _get_nc(cfg_key=()):
    if cfg_key not in _CACHE:
        cfg = Cfg()
        nc = build_nc(cfg)
        runner = make_runner(nc, cfg.NCORE)
        _CACHE[cfg_key] = (cfg, nc, runner)
    return _CACHE[cfg_key]


def kernel(**inputs):
    cfg, nc, runner = _get_nc()
    # structural preconditions from the generator: edges grouped by target,
    # exactly DEG edges per node; batch = repeat(arange(G), S)
    ei = np.asarray(inputs["edge_index"])
    N = cfg.NN * cfg.NCORE
    assert np.array_equal(ei[1], np.repeat(np.arange(N), cfg.DEG)), \
        "edge_index[1] must be repeat(arange(N), DEG)"
    maps = prep_inputs(inputs, cfg)
    results = runner(maps)
    outs = [results[c]["out4"].reshape(-1) for c in range(cfg.NCORE)]
    return np.concatenate(outs).astype(np.float32)


if __name__ == "__main__":
    cfg = Cfg()
    nc = build_nc(cfg)
    print("built + compiled OK")
